# revision 5
# baseline (speedup 1.0000x reference)
"""Fused band-attention transformer block on 8 Trainium2 NeuronCores.

Sharding: data-parallel over tokens (B*L = 8192 -> 1024 own tokens/core,
plus a 128-token sequence halo so window attention needs no collectives).
Per-core kernel computes rmsnorm -> QKV -> band attention -> O+residual ->
rmsnorm -> SwiGLU FFN -> residual, all activations feature-major (dim x tok),
matmuls in bf16 with f32 PSUM accumulation, residual stream in f32.
"""

from contextlib import ExitStack

import numpy as np
import ml_dtypes

import concourse.bacc as bacc
import concourse.bass as bass
import concourse.mybir as mybir
import concourse.tile as tile
from concourse.bass_utils import run_bass_kernel_spmd
from concourse.masks import make_identity

BF = ml_dtypes.bfloat16
F32 = mybir.dt.float32
BF16 = mybir.dt.bfloat16

B, L, DIM, H, W, DFF = 2, 4096, 2048, 16, 128, 8192
HD = DIM // H          # 128
P = 128
NCORES = 8
OWN = (B * L) // NCORES  # 1024 tokens per core
EXT = OWN + W            # 1152 with halo
KC = DIM // P            # 16 k-chunks over model dim
KF = DFF // P            # 64 k-chunks over ffn dim
NBLK = OWN // W          # 8 query blocks per core
EPS = 1e-6
SCALE = float(HD) ** -0.5

_CACHE = {}


def _build():
    nc = bacc.Bacc("TRN2", target_bir_lowering=False, debug=False)

    xT = nc.dram_tensor("xT", [DIM, EXT], F32, kind="ExternalInput")
    halo_kT = nc.dram_tensor("halo_kT", [P, H, W], BF16, kind="ExternalInput")
    halo_v = nc.dram_tensor("halo_v", [W, DIM], BF16, kind="ExternalInput")
    wq_tl = nc.dram_tensor("wq_tl", [KC, P, KC, P], BF16, kind="ExternalInput")
    wk_tl = nc.dram_tensor("wk_tl", [KC, P, KC, P], BF16, kind="ExternalInput")
    wv_tl = nc.dram_tensor("wv_tl", [8, P, KC, 256], BF16, kind="ExternalInput")
    wo_tl = nc.dram_tensor("wo_tl", [KC, P, KC, P], BF16, kind="ExternalInput")
    wfa_tl = nc.dram_tensor("wfa_tl", [KF, P, KC, P], BF16, kind="ExternalInput")
    wfc_tl = nc.dram_tensor("wfc_tl", [KF, P, KC, P], BF16, kind="ExternalInput")
    wfo_tl = nc.dram_tensor("wfo_tl", [KC, P, KF, P], BF16, kind="ExternalInput")
    yT = nc.dram_tensor("yT", [DIM, OWN], F32, kind="ExternalOutput")
    x2T_d = nc.dram_tensor("x2T_d", [DIM, OWN], F32)

    ext_cuts = [(0, 512), (512, 1024), (1024, EXT)]
    own_cuts = [(0, 512), (512, 1024)]

    with tile.TileContext(nc) as tc, ExitStack() as top:
        const = top.enter_context(tc.tile_pool(name="const", bufs=1))

        # band mask, additive: valid iff 1 <= j - p <= 128 (query p, window key j)
        mask = const.tile([P, 2 * W], F32)
        nc.gpsimd.memset(mask[:], 0.0)
        nc.gpsimd.affine_select(
            out=mask[:], in_=mask[:], compare_op=mybir.AluOpType.is_ge,
            fill=-1e4, base=-1, channel_multiplier=-1, pattern=[[1, 2 * W]])
        nc.gpsimd.affine_select(
            out=mask[:], in_=mask[:], compare_op=mybir.AluOpType.is_ge,
            fill=-1e4, base=W, channel_multiplier=1, pattern=[[-1, 2 * W]])

        ident = const.tile([P, P], BF16)
        make_identity(nc, ident[:])
        ones1 = const.tile([P, 1], BF16)
        nc.vector.memset(ones1[:], 1.0)
        eps_t = const.tile([1, 1], F32)
        nc.vector.memset(eps_t[:], EPS)

        rstd2_pool = top.enter_context(tc.tile_pool(name="rstd2p", bufs=1))
        rstd2_b = rstd2_pool.tile([P, OWN], F32, tag="rstd2_b")

        with ExitStack() as ao_scope:
            ao_pool = ao_scope.enter_context(tc.tile_pool(name="aop", bufs=1))

            with ExitStack() as mha:  # Ph1..Ph4 buffers
                kv_pool = mha.enter_context(tc.tile_pool(name="kv", bufs=1))
                kT = kv_pool.tile([P, H, EXT], BF16, tag="kT")
                vv = kv_pool.tile([P, EXT // P, DIM], BF16, tag="vv")
                t1_pool = mha.enter_context(tc.tile_pool(name="t1p", bufs=1))

                # ---- Phase 1: rmsnorm stats; t1 = x * rstd (g folded in W) ---
                t1 = []
                with ExitStack() as ph:
                    xf_pool = ph.enter_context(tc.tile_pool(name="xf", bufs=2))
                    xsq_pool = ph.enter_context(tc.tile_pool(name="xsq", bufs=2))
                    ss_ps = ph.enter_context(
                        tc.tile_pool(name="ss_ps", bufs=1, space="PSUM"))
                    ss = [ss_ps.tile([1, c1 - c0], F32, tag=f"ss{i}",
                                     name=f"ss{i}")
                          for i, (c0, c1) in enumerate(ext_cuts)]
                    for kc in range(KC):
                        xt = xf_pool.tile([P, EXT], F32, tag="xf", name="xf")
                        nc.sync.dma_start(out=xt[:],
                                          in_=xT[kc * P:(kc + 1) * P, :])
                        xsq = xsq_pool.tile([P, EXT], BF16, tag="xsq",
                                            name="xsq")
                        nc.vector.tensor_mul(xsq[:], xt[:], xt[:])
                        for i, (c0, c1) in enumerate(ext_cuts):
                            nc.tensor.matmul(ss[i][:], ones1[:], xsq[:, c0:c1],
                                             start=(kc == 0),
                                             stop=(kc == KC - 1),
                                             skip_group_check=True)
                    rstd1 = xf_pool.tile([1, EXT], F32, tag="rstd1")
                    for i, (c0, c1) in enumerate(ext_cuts):
                        nc.scalar.activation(
                            out=rstd1[:, c0:c1], in_=ss[i][:],
                            func=mybir.ActivationFunctionType.Sqrt,
                            bias=eps_t[:], scale=1.0 / DIM)
                    nc.vector.reciprocal(rstd1[:], rstd1[:])
                    rstd1_b = xf_pool.tile([P, EXT], F32, tag="rstd1_b")
                    nc.gpsimd.partition_broadcast(rstd1_b[:], rstd1[:])
                    for kc in range(KC):
                        xt = xf_pool.tile([P, EXT], F32, tag="xf", name="xf2")
                        nc.sync.dma_start(out=xt[:],
                                          in_=xT[kc * P:(kc + 1) * P, :])
                        tt = t1_pool.tile([P, EXT], BF16, tag=f"t1_{kc}",
                                          name=f"t1_{kc}")
                        nc.vector.tensor_mul(tt[:], xt[:], rstd1_b[:])
                        t1.append(tt)

                # ---- Phase 2: V = t1.T @ Wv  (tok x dim layout) --------------
                with ExitStack() as ph:
                    wv_pool = ph.enter_context(tc.tile_pool(name="wv", bufs=2))
                    ps_pool = ph.enter_context(
                        tc.tile_pool(name="v_ps", bufs=4, space="PSUM"))
                    for ncol in range(8):
                        wv_t = wv_pool.tile([P, KC, 256], BF16, tag="wv",
                                            name="wv")
                        nc.sync.dma_start(out=wv_t[:], in_=wv_tl[ncol])
                        for mc in range(EXT // P):
                            ps = ps_pool.tile([P, 256], F32, tag="vps",
                                              name="vps")
                            for kc in range(KC):
                                nc.tensor.matmul(
                                    ps[:], t1[kc][:, mc * P:(mc + 1) * P],
                                    wv_t[:, kc, :],
                                    start=(kc == 0), stop=(kc == KC - 1))
                            nc.scalar.copy(
                                vv[:, mc, ncol * 256:(ncol + 1) * 256], ps[:])
                    hv = wv_pool.tile([P, DIM], BF16, tag="halo_v")
                    nc.sync.dma_start(out=hv[:], in_=halo_v[:])
                    nc.vector.tensor_add(vv[:, 0, :], vv[:, 0, :], hv[:])

                # ---- Phase 3: K (dim x tok layout) ---------------------------
                with ExitStack() as ph:
                    w_pool = ph.enter_context(tc.tile_pool(name="wkp", bufs=3))
                    ps_pool = ph.enter_context(
                        tc.tile_pool(name="k_ps", bufs=4, space="PSUM"))
                    for oc in range(KC):
                        wk_t = w_pool.tile([P, KC, P], BF16, tag="wk",
                                           name="wk")
                        nc.sync.dma_start(out=wk_t[:], in_=wk_tl[oc])
                        for (c0, c1) in ext_cuts:
                            ps = ps_pool.tile([P, 512], F32, tag="kps",
                                              name="kps")
                            for kc in range(KC):
                                nc.tensor.matmul(
                                    ps[:, :c1 - c0], wk_t[:, kc, :],
                                    t1[kc][:, c0:c1],
                                    start=(kc == 0), stop=(kc == KC - 1))
                            nc.scalar.copy(kT[:, oc, c0:c1], ps[:, :c1 - c0])
                    hk = w_pool.tile([P, H, W], BF16, tag="halo_k")
                    nc.sync.dma_start(out=hk[:], in_=halo_kT[:])
                    nc.vector.tensor_add(kT[:, :, 0:W], kT[:, :, 0:W], hk[:])

                # ---- Phase 4: per head: Q then band attention ----------------
                aoT = ao_pool.tile([P, KC, OWN], BF16, tag="aoT")
                with ExitStack() as ph:
                    w_pool = ph.enter_context(tc.tile_pool(name="wqp", bufs=3))
                    qh_pool = ph.enter_context(tc.tile_pool(name="qhp", bufs=2))
                    sm_pool = ph.enter_context(tc.tile_pool(name="smp", bufs=3))
                    p_pool = ph.enter_context(tc.tile_pool(name="ppp", bufs=3))
                    q_ps = ph.enter_context(
                        tc.tile_pool(name="q_ps", bufs=2, space="PSUM"))
                    sc_ps = ph.enter_context(
                        tc.tile_pool(name="sc_ps", bufs=2, space="PSUM"))
                    tp_ps = ph.enter_context(
                        tc.tile_pool(name="tp_ps", bufs=2, space="PSUM"))
                    av_ps = ph.enter_context(
                        tc.tile_pool(name="av_ps", bufs=2, space="PSUM"))
                    for h in range(H):
                        wq_t = w_pool.tile([P, KC, P], BF16, tag="wq",
                                           name="wq")
                        nc.sync.dma_start(out=wq_t[:], in_=wq_tl[h])
                        qh = qh_pool.tile([P, OWN], BF16, tag="qh", name="qh")
                        for (c0, c1) in own_cuts:
                            ps = q_ps.tile([P, 512], F32, tag="qps", name="qps")
                            for kc in range(KC):
                                nc.tensor.matmul(
                                    ps[:], wq_t[:, kc, :],
                                    t1[kc][:, W + c0:W + c1],
                                    start=(kc == 0), stop=(kc == KC - 1))
                            nc.scalar.copy(qh[:, c0:c1], ps[:])
                        for n in range(NBLK):
                            sc = sc_ps.tile([P, 2 * W], F32, tag="sc",
                                            name="sc")
                            nc.tensor.matmul(sc[:], qh[:, n * W:(n + 1) * W],
                                             kT[:, h, n * W:n * W + 2 * W],
                                             start=True, stop=True)
                            sm = sm_pool.tile([P, 2 * W], F32, tag="sm",
                                              name="sm")
                            nc.vector.scalar_tensor_tensor(
                                out=sm[:], in0=sc[:], scalar=SCALE, in1=mask[:],
                                op0=mybir.AluOpType.mult,
                                op1=mybir.AluOpType.add)
                            pe = p_pool.tile([P, 2 * W], BF16, tag="pe",
                                             name="pe")
                            sumexp = sm_pool.tile([P, 1], F32, tag="sumexp",
                                                  name="sumexp")
                            nc.scalar.activation(
                                out=pe[:], in_=sm[:],
                                func=mybir.ActivationFunctionType.Exp,
                                accum_out=sumexp[:])
                            recip = sm_pool.tile([P, 1], F32, tag="recip",
                                                 name="recip")
                            nc.vector.reciprocal(recip[:], sumexp[:])
                            pn = p_pool.tile([P, 2 * W], BF16, tag="pn",
                                             name="pn")
                            nc.vector.tensor_scalar_mul(pn[:], pe[:], recip[:])
                            pT = p_pool.tile([P, 2, W], BF16, tag="pT",
                                             name="pT")
                            for c in range(2):
                                tp = tp_ps.tile([P, W], BF16, tag="tp",
                                                name="tp")
                                nc.tensor.transpose(
                                    tp[:], pn[:, c * W:(c + 1) * W], ident[:])
                                nc.vector.tensor_copy(pT[:, c, :], tp[:])
                            av = av_ps.tile([P, W], F32, tag="av", name="av")
                            for c in range(2):
                                nc.tensor.matmul(
                                    av[:], vv[:, n + c, h * HD:(h + 1) * HD],
                                    pT[:, c, :], start=(c == 0), stop=(c == 1))
                            nc.scalar.copy(aoT[:, h, n * W:(n + 1) * W], av[:])

            # ---- Phase 5: O proj + residual -> x2 (DRAM), rmsnorm2 stats -----
            with ExitStack() as ph:
                w_pool = ph.enter_context(tc.tile_pool(name="wop", bufs=3))
                st_pool = ph.enter_context(tc.tile_pool(name="ost", bufs=3))
                ps_pool = ph.enter_context(
                    tc.tile_pool(name="o_ps", bufs=3, space="PSUM"))
                ss_ps = ph.enter_context(
                    tc.tile_pool(name="ss2_ps", bufs=1, space="PSUM"))
                ss2 = [ss_ps.tile([1, 512], F32, tag=f"ss2_{i}",
                                  name=f"ss2_{i}") for i in range(2)]
                for oc in range(KC):
                    wo_t = w_pool.tile([P, KC, P], BF16, tag="wo", name="wo")
                    nc.sync.dma_start(out=wo_t[:], in_=wo_tl[oc])
                    for i, (c0, c1) in enumerate(own_cuts):
                        ps = ps_pool.tile([P, 512], F32, tag="ops", name="ops")
                        for kc in range(KC):
                            nc.tensor.matmul(ps[:], wo_t[:, kc, :],
                                             aoT[:, kc, c0:c1],
                                             start=(kc == 0),
                                             stop=(kc == KC - 1))
                        xres = st_pool.tile([P, 512], F32, tag="xres",
                                            name="xres")
                        nc.sync.dma_start(
                            out=xres[:],
                            in_=xT[oc * P:(oc + 1) * P, W + c0:W + c1])
                        x2 = st_pool.tile([P, 512], F32, tag="x2", name="x2")
                        nc.vector.tensor_add(x2[:], ps[:], xres[:])
                        nc.sync.dma_start(
                            out=x2T_d[oc * P:(oc + 1) * P, c0:c1], in_=x2[:])
                        xsq = st_pool.tile([P, 512], BF16, tag="xsq2",
                                           name="xsq2")
                        nc.vector.tensor_mul(xsq[:], x2[:], x2[:])
                        nc.tensor.matmul(ss2[i][:], ones1[:], xsq[:],
                                         start=(oc == 0), stop=(oc == KC - 1),
                                         skip_group_check=True)
                rstd2 = st_pool.tile([1, OWN], F32, tag="rstd2")
                for i, (c0, c1) in enumerate(own_cuts):
                    nc.scalar.activation(out=rstd2[:, c0:c1], in_=ss2[i][:],
                                         func=mybir.ActivationFunctionType.Sqrt,
                                         bias=eps_t[:], scale=1.0 / DIM)
                nc.vector.reciprocal(rstd2[:], rstd2[:])
                nc.gpsimd.partition_broadcast(rstd2_b[:], rstd2[:])

        # ---- Phase 6: SwiGLU FFN + residual ----------------------------------
        for (c0, c1) in own_cuts:
            with ExitStack() as ph:
                t2_pool = ph.enter_context(tc.tile_pool(name="t2p", bufs=1))
                x2s_pool = ph.enter_context(tc.tile_pool(name="x2s", bufs=3))
                h_pool = ph.enter_context(tc.tile_pool(name="hbufp", bufs=1))
                wf_pool = ph.enter_context(tc.tile_pool(name="wfp", bufs=3))
                wfo_pool = ph.enter_context(tc.tile_pool(name="wfop", bufs=2))
                s_pool = ph.enter_context(tc.tile_pool(name="silp", bufs=3))
                ps_pool = ph.enter_context(
                    tc.tile_pool(name="f_ps", bufs=2, space="PSUM"))

                t2 = []
                for kc in range(KC):
                    x2s = x2s_pool.tile([P, 512], F32, tag="x2l", name="x2l")
                    nc.sync.dma_start(out=x2s[:],
                                      in_=x2T_d[kc * P:(kc + 1) * P, c0:c1])
                    tt = t2_pool.tile([P, 512], BF16, tag=f"t2_{kc}",
                                      name=f"t2_{kc}")
                    nc.vector.tensor_mul(tt[:], x2s[:], rstd2_b[:, c0:c1])
                    t2.append(tt)

                hbuf = h_pool.tile([P, KF, 512], BF16)
                for oc in range(KF):
                    wfa_t = wf_pool.tile([P, KC, P], BF16, tag="wfa",
                                         name="wfa")
                    nc.sync.dma_start(out=wfa_t[:], in_=wfa_tl[oc])
                    ps1 = ps_pool.tile([P, 512], F32, tag="ps1", name="ps1")
                    for kc in range(KC):
                        nc.tensor.matmul(ps1[:], wfa_t[:, kc, :], t2[kc][:],
                                         start=(kc == 0), stop=(kc == KC - 1))
                    sil = s_pool.tile([P, 512], BF16, tag="sil", name="sil")
                    nc.scalar.activation(out=sil[:], in_=ps1[:],
                                         func=mybir.ActivationFunctionType.Silu)
                    wfc_t = wf_pool.tile([P, KC, P], BF16, tag="wfc",
                                         name="wfc")
                    nc.sync.dma_start(out=wfc_t[:], in_=wfc_tl[oc])
                    ps2 = ps_pool.tile([P, 512], F32, tag="ps2", name="ps2")
                    for kc in range(KC):
                        nc.tensor.matmul(ps2[:], wfc_t[:, kc, :], t2[kc][:],
                                         start=(kc == 0), stop=(kc == KC - 1))
                    nc.vector.tensor_mul(hbuf[:, oc, :], ps2[:], sil[:])

                for oc in range(KC):
                    wfo_t = wfo_pool.tile([P, KF, P], BF16, tag="wfo",
                                          name="wfo")
                    nc.sync.dma_start(out=wfo_t[:], in_=wfo_tl[oc])
                    ps = ps_pool.tile([P, 512], F32, tag="ps3", name="ps3")
                    for kc in range(KF):
                        nc.tensor.matmul(ps[:], wfo_t[:, kc, :],
                                         hbuf[:, kc, :],
                                         start=(kc == 0), stop=(kc == KF - 1))
                    x2s = x2s_pool.tile([P, 512], F32, tag="x2res",
                                        name="x2res")
                    nc.sync.dma_start(out=x2s[:],
                                      in_=x2T_d[oc * P:(oc + 1) * P, c0:c1])
                    yt = x2s_pool.tile([P, 512], F32, tag="yt", name="yt")
                    nc.vector.tensor_add(yt[:], ps[:], x2s[:])
                    nc.sync.dma_start(out=yT[oc * P:(oc + 1) * P, c0:c1],
                                      in_=yt[:])

    nc.compile()
    return nc


def _tile_w(wt, nkc, noc, ocw):
    """(din, dout) -> (dout//ocw, 128, din//128, ocw) so each [oc] is contiguous."""
    return np.ascontiguousarray(
        wt.reshape(nkc, P, noc, ocw).transpose(2, 1, 0, 3))


def _prep_inputs(x, wq, wk, wv, wo, last_k_init, last_v_init,
                 w_fc, w_fc_act, w_fc_out, g_mha, g_ffn):
    wq_t = ((wq * g_mha[None, :]).T).astype(BF)
    wk_t = ((wk * g_mha[None, :]).T).astype(BF)
    wv_t = ((wv * g_mha[None, :]).T).astype(BF)
    wo_t = wo.T.astype(BF)
    wfa_t = ((w_fc_act * g_ffn[None, :]).T).astype(BF)
    wfc_t = ((w_fc * g_ffn[None, :]).T).astype(BF)
    wfo_t = w_fc_out.T.astype(BF)

    shared = {
        "wq_tl": _tile_w(wq_t, KC, KC, P),
        "wk_tl": _tile_w(wk_t, KC, KC, P),
        "wv_tl": _tile_w(wv_t, KC, 8, 256),
        "wo_tl": _tile_w(wo_t, KC, KC, P),
        "wfa_tl": _tile_w(wfa_t, KC, KF, P),
        "wfc_tl": _tile_w(wfc_t, KC, KF, P),
        "wfo_tl": _tile_w(wfo_t, KF, KC, P),
    }

    # halo k/v for first-chunk cores, from last_k/v_init
    hk = np.zeros((W, H, HD), np.float32)
    hk[1:W] = last_k_init
    halo_kT0 = np.ascontiguousarray(hk.transpose(2, 1, 0)).astype(BF)  # (hd,h,j)
    hv = np.zeros((W, DIM), np.float32)
    hv[1:W] = last_v_init.reshape(W - 1, DIM)
    halo_v0 = hv.astype(BF)
    halo_kTz = np.zeros_like(halo_kT0)
    halo_vz = np.zeros_like(halo_v0)

    in_maps = []
    for c in range(NCORES):
        b, s = divmod(c * OWN, L)
        xe = np.zeros((EXT, DIM), np.float32)
        xe[W:] = x[b, s:s + OWN]
        if s > 0:
            xe[:W] = x[b, s - W:s]
        m = dict(shared)
        m["xT"] = np.ascontiguousarray(xe.T)
        m["halo_kT"] = halo_kT0 if s == 0 else halo_kTz
        m["halo_v"] = halo_v0 if s == 0 else halo_vz
        in_maps.append(m)
    return in_maps


def _run(inputs, trace=False):
    if "nc" not in _CACHE:
        _CACHE["nc"] = _build()
    nc = _CACHE["nc"]
    in_maps = _prep_inputs(**{k: np.asarray(v) for k, v in inputs.items()})
    res = run_bass_kernel_spmd(nc, in_maps, core_ids=list(range(NCORES)),
                               trace=trace)
    y = np.empty((B, L, DIM), np.float32)
    for c in range(NCORES):
        b, s = divmod(c * OWN, L)
        y[b, s:s + OWN] = res.results[c]["yT"].T
    return y, res


def kernel(**inputs):
    y, _ = _run(inputs, trace=False)
    return y


# revision 9
# speedup vs baseline: 24.8431x; 24.8431x over previous
"""Fused band-attention transformer block on 8 Trainium2 NeuronCores.

Sharding: data-parallel over tokens (B*L = 8192 -> 1024 own tokens/core,
plus a 128-token sequence halo so window attention needs no collectives;
batch 0 -> cores 0-3, batch 1 -> cores 4-7).
Per-core kernel computes rmsnorm -> QKV -> band attention -> O+residual ->
rmsnorm -> SwiGLU FFN -> residual, all activations feature-major (dim x tok),
matmuls in bf16 with f32 PSUM accumulation, residual stream in f32.
RMSNorm scales are folded into the matmul PSUM evictions (columns via a
partition-broadcast rstd row, V rows via a per-partition rstd column).
"""

from contextlib import ExitStack

import numpy as np
import ml_dtypes

import concourse.bacc as bacc
import concourse.bass as bass
import concourse.mybir as mybir
import concourse.tile as tile
from concourse.bass_utils import run_bass_kernel_spmd
from concourse.masks import make_identity

BF = ml_dtypes.bfloat16
F32 = mybir.dt.float32
BF16 = mybir.dt.bfloat16

B, L, DIM, H, W, DFF = 2, 4096, 2048, 16, 128, 8192
HD = DIM // H          # 128
P = 128
NCORES = 8
OWN = (B * L) // NCORES  # 1024 tokens per core
EXT = OWN + W            # 1152 with halo
KC = DIM // P            # 16 k-chunks over model dim
KF = DFF // P            # 64 k-chunks over ffn dim
NBLK = OWN // W          # 8 query blocks per core
MC = EXT // P            # 9 token tiles
EPS = 1e-6
SCALE = float(HD) ** -0.5

_CACHE = {}


def _build():
    nc = bacc.Bacc("TRN2", target_bir_lowering=False, debug=False)

    xT = nc.dram_tensor("xT", [DIM, EXT], F32, kind="ExternalInput")
    halo_kT = nc.dram_tensor("halo_kT", [P, H, W], BF16, kind="ExternalInput")
    halo_v = nc.dram_tensor("halo_v", [W, DIM], BF16, kind="ExternalInput")
    wq_tl = nc.dram_tensor("wq_tl", [KC, P, KC, P], BF16, kind="ExternalInput")
    wk_tl = nc.dram_tensor("wk_tl", [KC, P, KC, P], BF16, kind="ExternalInput")
    wv_tl = nc.dram_tensor("wv_tl", [4, P, KC, 512], BF16, kind="ExternalInput")
    wo_tl = nc.dram_tensor("wo_tl", [KC, P, KC, P], BF16, kind="ExternalInput")
    wfa_tl = nc.dram_tensor("wfa_tl", [KF, P, KC, P], BF16, kind="ExternalInput")
    wfc_tl = nc.dram_tensor("wfc_tl", [KF, P, KC, P], BF16, kind="ExternalInput")
    wfo_tl = nc.dram_tensor("wfo_tl", [KC, P, KF, P], BF16, kind="ExternalInput")
    yT = nc.dram_tensor("yT", [DIM, OWN], F32, kind="ExternalOutput")

    ext_cuts = [(0, 512), (512, 1024), (1024, EXT)]
    own_cuts = [(0, 512), (512, 1024)]

    with tile.TileContext(nc) as tc, ExitStack() as top:
        dram = top.enter_context(tc.tile_pool(name="dram", bufs=1, space="DRAM"))
        x2T_d = dram.tile([DIM, OWN], F32, tag="x2T_d")
        x2b_d = dram.tile([DIM, OWN], BF16, tag="x2b_d")
        rstd1_d = dram.tile([1, EXT], F32, tag="rstd1_d")

        const = top.enter_context(tc.tile_pool(name="const", bufs=1))

        # band mask, additive: valid iff 1 <= j - p <= 128 (query p, window key j)
        mask = const.tile([P, 2 * W], F32)
        nc.gpsimd.memset(mask[:], 0.0)
        nc.gpsimd.affine_select(
            out=mask[:], in_=mask[:], compare_op=mybir.AluOpType.is_ge,
            fill=-1e4, base=-1, channel_multiplier=-1, pattern=[[1, 2 * W]])
        nc.gpsimd.affine_select(
            out=mask[:], in_=mask[:], compare_op=mybir.AluOpType.is_ge,
            fill=-1e4, base=W, channel_multiplier=1, pattern=[[-1, 2 * W]])

        ident = const.tile([P, P], BF16)
        make_identity(nc, ident[:])
        ones1 = const.tile([P, 1], BF16)
        nc.vector.memset(ones1[:], 1.0)
        eps_t = const.tile([1, 1], F32)
        nc.vector.memset(eps_t[:], EPS)

        rstd2_pool = top.enter_context(tc.tile_pool(name="rstd2p", bufs=1))
        rstd2_b = rstd2_pool.tile([P, OWN], F32, tag="rstd2_b")

        if True:
            with ExitStack() as mha:  # Ph1..Ph5 buffers
                kv_pool = mha.enter_context(tc.tile_pool(name="kv", bufs=1))
                kT = kv_pool.tile([P, H, EXT], BF16, tag="kT")
                vv = kv_pool.tile([P, MC, DIM], BF16, tag="vv")
                xb_pool = mha.enter_context(tc.tile_pool(name="xbp", bufs=1))
                rs_pool = mha.enter_context(tc.tile_pool(name="rsp", bufs=1))
                rstd1_b = rs_pool.tile([P, EXT], F32, tag="rstd1_b")
                rstd1_c = rs_pool.tile([P, MC], F32, tag="rstd1_c")

                # ---- Phase 1: stream x, cast to bf16, rmsnorm1 stats ---------
                xb = []
                with ExitStack() as ph:
                    xf_pool = ph.enter_context(tc.tile_pool(name="xf", bufs=3))
                    xsq_pool = ph.enter_context(tc.tile_pool(name="xsq", bufs=3))
                    ss_ps = ph.enter_context(
                        tc.tile_pool(name="ss_ps", bufs=1, space="PSUM"))
                    ss = [ss_ps.tile([1, c1 - c0], F32, tag=f"ss{i}",
                                     name=f"ss{i}")
                          for i, (c0, c1) in enumerate(ext_cuts)]
                    for kc in range(KC):
                        xt = xf_pool.tile([P, EXT], F32, tag="xf", name="xf")
                        nc.sync.dma_start(out=xt[:],
                                          in_=xT[kc * P:(kc + 1) * P, :])
                        xbt = xb_pool.tile([P, EXT], BF16, tag=f"xb_{kc}",
                                           name=f"xb_{kc}")
                        nc.vector.tensor_copy(xbt[:], xt[:])
                        xb.append(xbt)
                        xsq = xsq_pool.tile([P, EXT], BF16, tag="xsq",
                                            name="xsq")
                        nc.vector.tensor_mul(xsq[:], xbt[:], xbt[:])
                        for i, (c0, c1) in enumerate(ext_cuts):
                            nc.tensor.matmul(ss[i][:], ones1[:], xsq[:, c0:c1],
                                             start=(kc == 0),
                                             stop=(kc == KC - 1),
                                             skip_group_check=True)
                    rstd1 = rs_pool.tile([1, EXT], F32, tag="rstd1")
                    for i, (c0, c1) in enumerate(ext_cuts):
                        nc.scalar.activation(
                            out=rstd1[:, c0:c1], in_=ss[i][:],
                            func=mybir.ActivationFunctionType.Sqrt,
                            bias=eps_t[:], scale=1.0 / DIM)
                    nc.vector.reciprocal(rstd1[:], rstd1[:])
                    nc.gpsimd.partition_broadcast(rstd1_b[:], rstd1[:])
                    # rstd as a (tok%128, tile) column view, via DRAM roundtrip
                    nc.sync.dma_start(out=rstd1_d[:], in_=rstd1[:])
                    nc.sync.dma_start(
                        out=rstd1_c[:],
                        in_=rstd1_d.rearrange("o (m p) -> (o p) m", p=P))

                # ---- Phase 2: V = (x.T @ Wv) * rstd  (tok x dim layout) ------
                with ExitStack() as ph:
                    wv_pool = ph.enter_context(tc.tile_pool(name="wv", bufs=2))
                    ps_pool = ph.enter_context(
                        tc.tile_pool(name="v_ps", bufs=4, space="PSUM"))
                    for ncol in range(4):
                        wv_t = wv_pool.tile([P, KC, 512], BF16, tag="wv",
                                            name="wv")
                        nc.sync.dma_start(out=wv_t[:], in_=wv_tl[ncol])
                        for mc in range(MC):
                            ps = ps_pool.tile([P, 512], F32, tag="vps",
                                              name="vps")
                            for kc in range(KC):
                                nc.tensor.matmul(
                                    ps[:], xb[kc][:, mc * P:(mc + 1) * P],
                                    wv_t[:, kc, :],
                                    start=(kc == 0), stop=(kc == KC - 1))
                            nc.vector.tensor_scalar_mul(
                                vv[:, mc, ncol * 512:(ncol + 1) * 512], ps[:],
                                rstd1_c[:, mc:mc + 1])
                    hv = wv_pool.tile([P, DIM], BF16, tag="halo_v")
                    nc.sync.dma_start(out=hv[:], in_=halo_v[:])
                    nc.vector.tensor_add(vv[:, 0, :], vv[:, 0, :], hv[:])

                # ---- Phase 3: K = (Wk.T @ x) * rstd (dim x tok layout) -------
                with ExitStack() as ph:
                    w_pool = ph.enter_context(tc.tile_pool(name="wkp", bufs=3))
                    ps_pool = ph.enter_context(
                        tc.tile_pool(name="k_ps", bufs=4, space="PSUM"))
                    for oc in range(KC):
                        wk_t = w_pool.tile([P, KC, P], BF16, tag="wk",
                                           name="wk")
                        nc.sync.dma_start(out=wk_t[:], in_=wk_tl[oc])
                        for (c0, c1) in ext_cuts:
                            ps = ps_pool.tile([P, 512], F32, tag="kps",
                                              name="kps")
                            for kc in range(KC):
                                nc.tensor.matmul(
                                    ps[:, :c1 - c0], wk_t[:, kc, :],
                                    xb[kc][:, c0:c1],
                                    start=(kc == 0), stop=(kc == KC - 1))
                            nc.vector.tensor_mul(kT[:, oc, c0:c1],
                                                 ps[:, :c1 - c0],
                                                 rstd1_b[:, c0:c1])
                    hk = w_pool.tile([P, H, W], BF16, tag="halo_k")
                    nc.sync.dma_start(out=hk[:], in_=halo_kT[:])
                    nc.vector.tensor_add(kT[:, :, 0:W], kT[:, :, 0:W], hk[:])

                # ---- Phase 4: per head: Q then band attention ----------------
                ao_pool = mha.enter_context(tc.tile_pool(name="aop", bufs=1))
                aoT = ao_pool.tile([P, KC, OWN], BF16, tag="aoT")
                with ExitStack() as ph:
                    w_pool = ph.enter_context(tc.tile_pool(name="wqp", bufs=3))
                    qh_pool = ph.enter_context(tc.tile_pool(name="qhp", bufs=2))
                    sm_pool = ph.enter_context(tc.tile_pool(name="smp", bufs=3))
                    p_pool = ph.enter_context(tc.tile_pool(name="ppp", bufs=3))
                    q_ps = ph.enter_context(
                        tc.tile_pool(name="q_ps", bufs=2, space="PSUM"))
                    sc_ps = ph.enter_context(
                        tc.tile_pool(name="sc_ps", bufs=2, space="PSUM"))
                    tp_ps = ph.enter_context(
                        tc.tile_pool(name="tp_ps", bufs=2, space="PSUM"))
                    av_ps = ph.enter_context(
                        tc.tile_pool(name="av_ps", bufs=2, space="PSUM"))
                    for h in range(H):
                        wq_t = w_pool.tile([P, KC, P], BF16, tag="wq",
                                           name="wq")
                        nc.sync.dma_start(out=wq_t[:], in_=wq_tl[h])
                        qh = qh_pool.tile([P, OWN], BF16, tag="qh", name="qh")
                        for (c0, c1) in own_cuts:
                            ps = q_ps.tile([P, 512], F32, tag="qps", name="qps")
                            for kc in range(KC):
                                nc.tensor.matmul(
                                    ps[:], wq_t[:, kc, :],
                                    xb[kc][:, W + c0:W + c1],
                                    start=(kc == 0), stop=(kc == KC - 1))
                            nc.vector.tensor_mul(qh[:, c0:c1], ps[:],
                                                 rstd1_b[:, W + c0:W + c1])
                        for n in range(NBLK):
                            sc = sc_ps.tile([P, 2 * W], F32, tag="sc",
                                            name="sc")
                            nc.tensor.matmul(sc[:], qh[:, n * W:(n + 1) * W],
                                             kT[:, h, n * W:n * W + 2 * W],
                                             start=True, stop=True)
                            sm = sm_pool.tile([P, 2 * W], F32, tag="sm",
                                              name="sm")
                            nc.vector.scalar_tensor_tensor(
                                out=sm[:], in0=sc[:], scalar=SCALE, in1=mask[:],
                                op0=mybir.AluOpType.mult,
                                op1=mybir.AluOpType.add)
                            pe = p_pool.tile([P, 2 * W], BF16, tag="pe",
                                             name="pe")
                            sumexp = sm_pool.tile([P, 1], F32, tag="sumexp",
                                                  name="sumexp")
                            nc.scalar.activation(
                                out=pe[:], in_=sm[:],
                                func=mybir.ActivationFunctionType.Exp,
                                accum_out=sumexp[:])
                            recip = sm_pool.tile([P, 1], F32, tag="recip",
                                                 name="recip")
                            nc.vector.reciprocal(recip[:], sumexp[:])
                            pn = p_pool.tile([P, 2 * W], BF16, tag="pn",
                                             name="pn")
                            nc.vector.tensor_scalar_mul(pn[:], pe[:], recip[:])
                            pT = p_pool.tile([P, 2, W], BF16, tag="pT",
                                             name="pT")
                            for c in range(2):
                                tp = tp_ps.tile([P, W], BF16, tag="tp",
                                                name="tp")
                                nc.tensor.transpose(
                                    tp[:], pn[:, c * W:(c + 1) * W], ident[:])
                                nc.vector.tensor_copy(pT[:, c, :], tp[:])
                            av = av_ps.tile([P, W], F32, tag="av", name="av")
                            for c in range(2):
                                nc.tensor.matmul(
                                    av[:], vv[:, n + c, h * HD:(h + 1) * HD],
                                    pT[:, c, :], start=(c == 0), stop=(c == 1))
                            nc.scalar.copy(aoT[:, h, n * W:(n + 1) * W], av[:])

                # ---- Phase 5: O proj + residual -> x2 (DRAM f32+bf16), ------
                # ---- fused rmsnorm2 stats -----------------------------------
                with ExitStack() as ph:
                    w_pool = ph.enter_context(tc.tile_pool(name="wop", bufs=3))
                    st_pool = ph.enter_context(tc.tile_pool(name="ost", bufs=3))
                    ps_pool = ph.enter_context(
                        tc.tile_pool(name="o_ps", bufs=3, space="PSUM"))
                    ss_ps = ph.enter_context(
                        tc.tile_pool(name="ss2_ps", bufs=1, space="PSUM"))
                    ss2 = [ss_ps.tile([1, 512], F32, tag=f"ss2_{i}",
                                      name=f"ss2_{i}") for i in range(2)]
                    for oc in range(KC):
                        wo_t = w_pool.tile([P, KC, P], BF16, tag="wo", name="wo")
                        nc.sync.dma_start(out=wo_t[:], in_=wo_tl[oc])
                        for i, (c0, c1) in enumerate(own_cuts):
                            ps = ps_pool.tile([P, 512], F32, tag="ops", name="ops")
                            for kc in range(KC):
                                nc.tensor.matmul(ps[:], wo_t[:, kc, :],
                                                 aoT[:, kc, c0:c1],
                                                 start=(kc == 0),
                                                 stop=(kc == KC - 1))
                            xres = st_pool.tile([P, 512], F32, tag="xres",
                                                name="xres")
                            nc.sync.dma_start(
                                out=xres[:],
                                in_=xT[oc * P:(oc + 1) * P, W + c0:W + c1])
                            x2 = st_pool.tile([P, 512], F32, tag="x2", name="x2")
                            nc.vector.tensor_add(x2[:], ps[:], xres[:])
                            nc.sync.dma_start(
                                out=x2T_d[oc * P:(oc + 1) * P, c0:c1], in_=x2[:])
                            xb2s = st_pool.tile([P, 512], BF16, tag="xb2s",
                                                name="xb2s")
                            nc.scalar.copy(xb2s[:], x2[:])
                            nc.sync.dma_start(
                                out=x2b_d[oc * P:(oc + 1) * P, c0:c1],
                                in_=xb2s[:])
                            xsq = st_pool.tile([P, 512], BF16, tag="xsq2",
                                               name="xsq2")
                            nc.vector.tensor_mul(xsq[:], x2[:], x2[:])
                            nc.tensor.matmul(ss2[i][:], ones1[:], xsq[:],
                                             start=(oc == 0), stop=(oc == KC - 1),
                                             skip_group_check=True)
                    rstd2 = st_pool.tile([1, OWN], F32, tag="rstd2")
                    for i, (c0, c1) in enumerate(own_cuts):
                        nc.scalar.activation(out=rstd2[:, c0:c1], in_=ss2[i][:],
                                             func=mybir.ActivationFunctionType.Sqrt,
                                             bias=eps_t[:], scale=1.0 / DIM)
                    nc.vector.reciprocal(rstd2[:], rstd2[:])
                    nc.gpsimd.partition_broadcast(rstd2_b[:], rstd2[:])

        # ---- Phase 6: SwiGLU FFN + residual ----------------------------------
        for (c0, c1) in own_cuts:
            with ExitStack() as ph:
                t2_pool = ph.enter_context(tc.tile_pool(name="t2p", bufs=1))
                x2s_pool = ph.enter_context(tc.tile_pool(name="x2s", bufs=3))
                h_pool = ph.enter_context(tc.tile_pool(name="hbufp", bufs=1))
                wf_pool = ph.enter_context(tc.tile_pool(name="wfp", bufs=3))
                wfo_pool = ph.enter_context(tc.tile_pool(name="wfop", bufs=2))
                s_pool = ph.enter_context(tc.tile_pool(name="silp", bufs=3))
                ps_pool = ph.enter_context(
                    tc.tile_pool(name="f_ps", bufs=2, space="PSUM"))

                t2 = []
                for kc in range(KC):
                    xbs = x2s_pool.tile([P, 512], BF16, tag="xbs", name="xbs")
                    nc.sync.dma_start(out=xbs[:],
                                      in_=x2b_d[kc * P:(kc + 1) * P, c0:c1])
                    tt = t2_pool.tile([P, 512], BF16, tag=f"t2_{kc}",
                                      name=f"t2_{kc}")
                    nc.vector.tensor_mul(tt[:], xbs[:], rstd2_b[:, c0:c1])
                    t2.append(tt)

                hbuf = h_pool.tile([P, KF, 512], BF16)
                for oc in range(KF):
                    wfa_t = wf_pool.tile([P, KC, P], BF16, tag="wfa",
                                         name="wfa")
                    nc.sync.dma_start(out=wfa_t[:], in_=wfa_tl[oc])
                    ps1 = ps_pool.tile([P, 512], F32, tag="ps1", name="ps1")
                    for kc in range(KC):
                        nc.tensor.matmul(ps1[:], wfa_t[:, kc, :], t2[kc][:],
                                         start=(kc == 0), stop=(kc == KC - 1))
                    sil = s_pool.tile([P, 512], BF16, tag="sil", name="sil")
                    nc.scalar.activation(out=sil[:], in_=ps1[:],
                                         func=mybir.ActivationFunctionType.Silu)
                    wfc_t = wf_pool.tile([P, KC, P], BF16, tag="wfc",
                                         name="wfc")
                    nc.sync.dma_start(out=wfc_t[:], in_=wfc_tl[oc])
                    ps2 = ps_pool.tile([P, 512], F32, tag="ps2", name="ps2")
                    for kc in range(KC):
                        nc.tensor.matmul(ps2[:], wfc_t[:, kc, :], t2[kc][:],
                                         start=(kc == 0), stop=(kc == KC - 1))
                    nc.vector.tensor_mul(hbuf[:, oc, :], ps2[:], sil[:])

                for oc in range(KC):
                    wfo_t = wfo_pool.tile([P, KF, P], BF16, tag="wfo",
                                          name="wfo")
                    nc.sync.dma_start(out=wfo_t[:], in_=wfo_tl[oc])
                    ps = ps_pool.tile([P, 512], F32, tag="ps3", name="ps3")
                    for kc in range(KF):
                        nc.tensor.matmul(ps[:], wfo_t[:, kc, :],
                                         hbuf[:, kc, :],
                                         start=(kc == 0), stop=(kc == KF - 1))
                    x2s = x2s_pool.tile([P, 512], F32, tag="x2res",
                                        name="x2res")
                    nc.sync.dma_start(out=x2s[:],
                                      in_=x2T_d[oc * P:(oc + 1) * P, c0:c1])
                    yt = x2s_pool.tile([P, 512], F32, tag="yt", name="yt")
                    nc.vector.tensor_add(yt[:], ps[:], x2s[:])
                    nc.sync.dma_start(out=yT[oc * P:(oc + 1) * P, c0:c1],
                                      in_=yt[:])

    nc.compile()
    return nc


def _tile_w(wt, nkc, noc, ocw):
    """(din, dout) -> (dout//ocw, 128, din//128, ocw) so each [oc] is contiguous."""
    return np.ascontiguousarray(
        wt.reshape(nkc, P, noc, ocw).transpose(2, 1, 0, 3))


def _prep_inputs(x, wq, wk, wv, wo, last_k_init, last_v_init,
                 w_fc, w_fc_act, w_fc_out, g_mha, g_ffn):
    wq_t = ((wq * g_mha[None, :]).T).astype(BF)
    wk_t = ((wk * g_mha[None, :]).T).astype(BF)
    wv_t = ((wv * g_mha[None, :]).T).astype(BF)
    wo_t = wo.T.astype(BF)
    wfa_t = ((w_fc_act * g_ffn[None, :]).T).astype(BF)
    wfc_t = ((w_fc * g_ffn[None, :]).T).astype(BF)
    wfo_t = w_fc_out.T.astype(BF)

    shared = {
        "wq_tl": _tile_w(wq_t, KC, KC, P),
        "wk_tl": _tile_w(wk_t, KC, KC, P),
        "wv_tl": _tile_w(wv_t, KC, 4, 512),
        "wo_tl": _tile_w(wo_t, KC, KC, P),
        "wfa_tl": _tile_w(wfa_t, KC, KF, P),
        "wfc_tl": _tile_w(wfc_t, KC, KF, P),
        "wfo_tl": _tile_w(wfo_t, KF, KC, P),
    }

    # halo k/v for first-chunk cores, from last_k/v_init
    hk = np.zeros((W, H, HD), np.float32)
    hk[1:W] = last_k_init
    halo_kT0 = np.ascontiguousarray(hk.transpose(2, 1, 0)).astype(BF)  # (hd,h,j)
    hv = np.zeros((W, DIM), np.float32)
    hv[1:W] = last_v_init.reshape(W - 1, DIM)
    halo_v0 = hv.astype(BF)
    halo_kTz = np.zeros_like(halo_kT0)
    halo_vz = np.zeros_like(halo_v0)

    in_maps = []
    for c in range(NCORES):
        b, s = divmod(c * OWN, L)
        xe = np.zeros((EXT, DIM), np.float32)
        xe[W:] = x[b, s:s + OWN]
        if s > 0:
            xe[:W] = x[b, s - W:s]
        m = dict(shared)
        m["xT"] = np.ascontiguousarray(xe.T)
        m["halo_kT"] = halo_kT0 if s == 0 else halo_kTz
        m["halo_v"] = halo_v0 if s == 0 else halo_vz
        in_maps.append(m)
    return in_maps


def _run(inputs, trace=False):
    if "nc" not in _CACHE:
        _CACHE["nc"] = _build()
    nc = _CACHE["nc"]
    in_maps = _prep_inputs(**{k: np.asarray(v) for k, v in inputs.items()})
    res = run_bass_kernel_spmd(nc, in_maps, core_ids=list(range(NCORES)),
                               trace=trace)
    y = np.empty((B, L, DIM), np.float32)
    for c in range(NCORES):
        b, s = divmod(c * OWN, L)
        y[b, s:s + OWN] = res.results[c]["yT"].T
    return y, res


def kernel(**inputs):
    y, _ = _run(inputs, trace=False)
    return y


# revision 10
# speedup vs baseline: 91.3372x; 3.6766x over previous
"""Fused band-attention transformer block on 8 Trainium2 NeuronCores.

Sharding: data-parallel over tokens (B*L = 8192 -> 1024 own tokens/core,
plus a 128-token sequence halo so window attention needs no collectives;
batch 0 -> cores 0-3, batch 1 -> cores 4-7).
Per-core kernel computes rmsnorm -> QKV -> band attention -> O+residual ->
rmsnorm -> SwiGLU FFN -> residual, all activations feature-major (dim x tok),
matmuls in bf16 with f32 PSUM accumulation, residual stream in f32.
RMSNorm scales are folded into the matmul PSUM evictions (columns via a
partition-broadcast rstd row, V rows via a per-partition rstd column).
"""

from contextlib import ExitStack, nullcontext

import numpy as np
import ml_dtypes

import concourse.bacc as bacc
import concourse.bass as bass
import concourse.mybir as mybir
import concourse.tile as tile
from concourse.bass_utils import run_bass_kernel_spmd
from concourse.masks import make_identity

BF = ml_dtypes.bfloat16
F32 = mybir.dt.float32
BF16 = mybir.dt.bfloat16

B, L, DIM, H, W, DFF = 2, 4096, 2048, 16, 128, 8192
HD = DIM // H          # 128
P = 128
NCORES = 8
OWN = (B * L) // NCORES  # 1024 tokens per core
EXT = OWN + W            # 1152 with halo
KC = DIM // P            # 16 k-chunks over model dim
KF = DFF // P            # 64 k-chunks over ffn dim
NBLK = OWN // W          # 8 query blocks per core
MC = EXT // P            # 9 token tiles
EPS = 1e-6
SCALE = float(HD) ** -0.5

_CACHE = {}


def _build(n_loop=1):
    nc = bacc.Bacc("TRN2", target_bir_lowering=False, debug=False)

    xT = nc.dram_tensor("xT", [DIM, EXT], F32, kind="ExternalInput")
    halo_kT = nc.dram_tensor("halo_kT", [P, H, W], BF16, kind="ExternalInput")
    halo_v = nc.dram_tensor("halo_v", [W, DIM], BF16, kind="ExternalInput")
    wq_tl = nc.dram_tensor("wq_tl", [KC, P, KC, P], BF16, kind="ExternalInput")
    wk_tl = nc.dram_tensor("wk_tl", [KC, P, KC, P], BF16, kind="ExternalInput")
    wv_tl = nc.dram_tensor("wv_tl", [4, P, KC, 512], BF16, kind="ExternalInput")
    wo_tl = nc.dram_tensor("wo_tl", [KC, P, KC, P], BF16, kind="ExternalInput")
    wfa_tl = nc.dram_tensor("wfa_tl", [KF, P, KC, P], BF16, kind="ExternalInput")
    wfc_tl = nc.dram_tensor("wfc_tl", [KF, P, KC, P], BF16, kind="ExternalInput")
    wfo_tl = nc.dram_tensor("wfo_tl", [KC, P, KF, P], BF16, kind="ExternalInput")
    yT = nc.dram_tensor("yT", [DIM, OWN], F32, kind="ExternalOutput")

    ext_cuts = [(0, 512), (512, 1024), (1024, EXT)]
    own_cuts = [(0, 512), (512, 1024)]

    with tile.TileContext(nc) as tc, ExitStack() as top:
        dram = top.enter_context(tc.tile_pool(name="dram", bufs=1, space="DRAM"))
        x2T_d = dram.tile([DIM, OWN], F32, tag="x2T_d")
        x2b_d = dram.tile([DIM, OWN], BF16, tag="x2b_d")
        rstd1_d = dram.tile([1, EXT], F32, tag="rstd1_d")

        const = top.enter_context(tc.tile_pool(name="const", bufs=1))

        # band mask, additive: valid iff 1 <= j - p <= 128 (query p, window key j)
        mask = const.tile([P, 2 * W], F32)
        nc.gpsimd.memset(mask[:], 0.0)
        nc.gpsimd.affine_select(
            out=mask[:], in_=mask[:], compare_op=mybir.AluOpType.is_ge,
            fill=-1e4, base=-1, channel_multiplier=-1, pattern=[[1, 2 * W]])
        nc.gpsimd.affine_select(
            out=mask[:], in_=mask[:], compare_op=mybir.AluOpType.is_ge,
            fill=-1e4, base=W, channel_multiplier=1, pattern=[[-1, 2 * W]])

        ident = const.tile([P, P], BF16)
        make_identity(nc, ident[:])
        ones1 = const.tile([P, 1], BF16)
        nc.vector.memset(ones1[:], 1.0)
        eps_t = const.tile([1, 1], F32)
        nc.vector.memset(eps_t[:], EPS)

        rstd2_pool = top.enter_context(tc.tile_pool(name="rstd2p", bufs=1))
        rstd2_b = rstd2_pool.tile([P, OWN], F32, tag="rstd2_b")

        with (tc.For_i(0, n_loop, 1) if n_loop > 1 else nullcontext()):
            with ExitStack() as mha:  # Ph1..Ph5 buffers
                kv_pool = mha.enter_context(tc.tile_pool(name="kv", bufs=1))
                kT = kv_pool.tile([P, H, EXT], BF16, tag="kT")
                vv = kv_pool.tile([P, MC, DIM], BF16, tag="vv")
                xb_pool = mha.enter_context(tc.tile_pool(name="xbp", bufs=1))
                rs_pool = mha.enter_context(tc.tile_pool(name="rsp", bufs=1))
                rstd1_b = rs_pool.tile([P, EXT], F32, tag="rstd1_b")
                rstd1_c = rs_pool.tile([P, MC], F32, tag="rstd1_c")

                # ---- Phase 1: stream x, cast to bf16, rmsnorm1 stats ---------
                xb = []
                with ExitStack() as ph:
                    xf_pool = ph.enter_context(tc.tile_pool(name="xf", bufs=3))
                    xsq_pool = ph.enter_context(tc.tile_pool(name="xsq", bufs=3))
                    ss_ps = ph.enter_context(
                        tc.tile_pool(name="ss_ps", bufs=1, space="PSUM"))
                    ss = [ss_ps.tile([1, c1 - c0], F32, tag=f"ss{i}",
                                     name=f"ss{i}")
                          for i, (c0, c1) in enumerate(ext_cuts)]
                    for kc in range(KC):
                        xt = xf_pool.tile([P, EXT], F32, tag="xf", name="xf")
                        nc.sync.dma_start(out=xt[:],
                                          in_=xT[kc * P:(kc + 1) * P, :])
                        xbt = xb_pool.tile([P, EXT], BF16, tag=f"xb_{kc}",
                                           name=f"xb_{kc}")
                        nc.vector.tensor_copy(xbt[:], xt[:])
                        xb.append(xbt)
                        xsq = xsq_pool.tile([P, EXT], BF16, tag="xsq",
                                            name="xsq")
                        nc.vector.tensor_mul(xsq[:], xbt[:], xbt[:])
                        for i, (c0, c1) in enumerate(ext_cuts):
                            nc.tensor.matmul(ss[i][:], ones1[:], xsq[:, c0:c1],
                                             start=(kc == 0),
                                             stop=(kc == KC - 1),
                                             skip_group_check=True)
                    rstd1 = rs_pool.tile([1, EXT], F32, tag="rstd1")
                    for i, (c0, c1) in enumerate(ext_cuts):
                        nc.scalar.activation(
                            out=rstd1[:, c0:c1], in_=ss[i][:],
                            func=mybir.ActivationFunctionType.Sqrt,
                            bias=eps_t[:], scale=1.0 / DIM)
                    nc.vector.reciprocal(rstd1[:], rstd1[:])
                    nc.gpsimd.partition_broadcast(rstd1_b[:], rstd1[:])
                    # rstd as a (tok%128, tile) column view, via DRAM roundtrip
                    nc.sync.dma_start(out=rstd1_d[:], in_=rstd1[:])
                    nc.sync.dma_start(
                        out=rstd1_c[:],
                        in_=rstd1_d.rearrange("o (m p) -> (o p) m", p=P))

                # ---- Phase 2: V = (x.T @ Wv) * rstd  (tok x dim layout) ------
                with ExitStack() as ph:
                    wv_pool = ph.enter_context(tc.tile_pool(name="wv", bufs=2))
                    ps_pool = ph.enter_context(
                        tc.tile_pool(name="v_ps", bufs=4, space="PSUM"))
                    for ncol in range(4):
                        wv_t = wv_pool.tile([P, KC, 512], BF16, tag="wv",
                                            name="wv")
                        nc.sync.dma_start(out=wv_t[:], in_=wv_tl[ncol])
                        for mc in range(MC):
                            ps = ps_pool.tile([P, 512], F32, tag="vps",
                                              name="vps")
                            for kc in range(KC):
                                nc.tensor.matmul(
                                    ps[:], xb[kc][:, mc * P:(mc + 1) * P],
                                    wv_t[:, kc, :],
                                    start=(kc == 0), stop=(kc == KC - 1))
                            nc.vector.tensor_scalar_mul(
                                vv[:, mc, ncol * 512:(ncol + 1) * 512], ps[:],
                                rstd1_c[:, mc:mc + 1])
                    hv = wv_pool.tile([P, DIM], BF16, tag="halo_v")
                    nc.sync.dma_start(out=hv[:], in_=halo_v[:])
                    nc.vector.tensor_add(vv[:, 0, :], vv[:, 0, :], hv[:])

                # ---- Phase 3: K = (Wk.T @ x) * rstd (dim x tok layout) -------
                with ExitStack() as ph:
                    w_pool = ph.enter_context(tc.tile_pool(name="wkp", bufs=3))
                    ps_pool = ph.enter_context(
                        tc.tile_pool(name="k_ps", bufs=4, space="PSUM"))
                    for oc in range(KC):
                        wk_t = w_pool.tile([P, KC, P], BF16, tag="wk",
                                           name="wk")
                        nc.sync.dma_start(out=wk_t[:], in_=wk_tl[oc])
                        for (c0, c1) in ext_cuts:
                            ps = ps_pool.tile([P, 512], F32, tag="kps",
                                              name="kps")
                            for kc in range(KC):
                                nc.tensor.matmul(
                                    ps[:, :c1 - c0], wk_t[:, kc, :],
                                    xb[kc][:, c0:c1],
                                    start=(kc == 0), stop=(kc == KC - 1))
                            nc.vector.tensor_mul(kT[:, oc, c0:c1],
                                                 ps[:, :c1 - c0],
                                                 rstd1_b[:, c0:c1])
                    hk = w_pool.tile([P, H, W], BF16, tag="halo_k")
                    nc.sync.dma_start(out=hk[:], in_=halo_kT[:])
                    nc.vector.tensor_add(kT[:, :, 0:W], kT[:, :, 0:W], hk[:])

                # ---- Phase 4: per head: Q then band attention ----------------
                ao_pool = mha.enter_context(tc.tile_pool(name="aop", bufs=1))
                aoT = ao_pool.tile([P, KC, OWN], BF16, tag="aoT")
                with ExitStack() as ph:
                    w_pool = ph.enter_context(tc.tile_pool(name="wqp", bufs=3))
                    qh_pool = ph.enter_context(tc.tile_pool(name="qhp", bufs=2))
                    sm_pool = ph.enter_context(tc.tile_pool(name="smp", bufs=3))
                    p_pool = ph.enter_context(tc.tile_pool(name="ppp", bufs=3))
                    q_ps = ph.enter_context(
                        tc.tile_pool(name="q_ps", bufs=2, space="PSUM"))
                    sc_ps = ph.enter_context(
                        tc.tile_pool(name="sc_ps", bufs=2, space="PSUM"))
                    tp_ps = ph.enter_context(
                        tc.tile_pool(name="tp_ps", bufs=2, space="PSUM"))
                    av_ps = ph.enter_context(
                        tc.tile_pool(name="av_ps", bufs=2, space="PSUM"))
                    for h in range(H):
                        wq_t = w_pool.tile([P, KC, P], BF16, tag="wq",
                                           name="wq")
                        nc.sync.dma_start(out=wq_t[:], in_=wq_tl[h])
                        qh = qh_pool.tile([P, OWN], BF16, tag="qh", name="qh")
                        for (c0, c1) in own_cuts:
                            ps = q_ps.tile([P, 512], F32, tag="qps", name="qps")
                            for kc in range(KC):
                                nc.tensor.matmul(
                                    ps[:], wq_t[:, kc, :],
                                    xb[kc][:, W + c0:W + c1],
                                    start=(kc == 0), stop=(kc == KC - 1))
                            nc.vector.tensor_mul(qh[:, c0:c1], ps[:],
                                                 rstd1_b[:, W + c0:W + c1])
                        for n in range(NBLK):
                            sc = sc_ps.tile([P, 2 * W], F32, tag="sc",
                                            name="sc")
                            nc.tensor.matmul(sc[:], qh[:, n * W:(n + 1) * W],
                                             kT[:, h, n * W:n * W + 2 * W],
                                             start=True, stop=True)
                            sm = sm_pool.tile([P, 2 * W], F32, tag="sm",
                                              name="sm")
                            nc.vector.scalar_tensor_tensor(
                                out=sm[:], in0=sc[:], scalar=SCALE, in1=mask[:],
                                op0=mybir.AluOpType.mult,
                                op1=mybir.AluOpType.add)
                            pe = p_pool.tile([P, 2 * W], BF16, tag="pe",
                                             name="pe")
                            sumexp = sm_pool.tile([P, 1], F32, tag="sumexp",
                                                  name="sumexp")
                            nc.scalar.activation(
                                out=pe[:], in_=sm[:],
                                func=mybir.ActivationFunctionType.Exp,
                                accum_out=sumexp[:])
                            recip = sm_pool.tile([P, 1], F32, tag="recip",
                                                 name="recip")
                            nc.vector.reciprocal(recip[:], sumexp[:])
                            pn = p_pool.tile([P, 2 * W], BF16, tag="pn",
                                             name="pn")
                            nc.vector.tensor_scalar_mul(pn[:], pe[:], recip[:])
                            pT = p_pool.tile([P, 2, W], BF16, tag="pT",
                                             name="pT")
                            for c in range(2):
                                tp = tp_ps.tile([P, W], BF16, tag="tp",
                                                name="tp")
                                nc.tensor.transpose(
                                    tp[:], pn[:, c * W:(c + 1) * W], ident[:])
                                nc.vector.tensor_copy(pT[:, c, :], tp[:])
                            av = av_ps.tile([P, W], F32, tag="av", name="av")
                            for c in range(2):
                                nc.tensor.matmul(
                                    av[:], vv[:, n + c, h * HD:(h + 1) * HD],
                                    pT[:, c, :], start=(c == 0), stop=(c == 1))
                            nc.scalar.copy(aoT[:, h, n * W:(n + 1) * W], av[:])

                # ---- Phase 5: O proj + residual -> x2 (DRAM f32+bf16), ------
                # ---- fused rmsnorm2 stats -----------------------------------
                with ExitStack() as ph:
                    w_pool = ph.enter_context(tc.tile_pool(name="wop", bufs=3))
                    st_pool = ph.enter_context(tc.tile_pool(name="ost", bufs=3))
                    ps_pool = ph.enter_context(
                        tc.tile_pool(name="o_ps", bufs=3, space="PSUM"))
                    ss_ps = ph.enter_context(
                        tc.tile_pool(name="ss2_ps", bufs=1, space="PSUM"))
                    ss2 = [ss_ps.tile([1, 512], F32, tag=f"ss2_{i}",
                                      name=f"ss2_{i}") for i in range(2)]
                    for oc in range(KC):
                        wo_t = w_pool.tile([P, KC, P], BF16, tag="wo", name="wo")
                        nc.sync.dma_start(out=wo_t[:], in_=wo_tl[oc])
                        for i, (c0, c1) in enumerate(own_cuts):
                            ps = ps_pool.tile([P, 512], F32, tag="ops", name="ops")
                            for kc in range(KC):
                                nc.tensor.matmul(ps[:], wo_t[:, kc, :],
                                                 aoT[:, kc, c0:c1],
                                                 start=(kc == 0),
                                                 stop=(kc == KC - 1))
                            xres = st_pool.tile([P, 512], F32, tag="xres",
                                                name="xres")
                            nc.sync.dma_start(
                                out=xres[:],
                                in_=xT[oc * P:(oc + 1) * P, W + c0:W + c1])
                            x2 = st_pool.tile([P, 512], F32, tag="x2", name="x2")
                            nc.vector.tensor_add(x2[:], ps[:], xres[:])
                            nc.sync.dma_start(
                                out=x2T_d[oc * P:(oc + 1) * P, c0:c1], in_=x2[:])
                            xb2s = st_pool.tile([P, 512], BF16, tag="xb2s",
                                                name="xb2s")
                            nc.scalar.copy(xb2s[:], x2[:])
                            nc.sync.dma_start(
                                out=x2b_d[oc * P:(oc + 1) * P, c0:c1],
                                in_=xb2s[:])
                            xsq = st_pool.tile([P, 512], BF16, tag="xsq2",
                                               name="xsq2")
                            nc.vector.tensor_mul(xsq[:], x2[:], x2[:])
                            nc.tensor.matmul(ss2[i][:], ones1[:], xsq[:],
                                             start=(oc == 0), stop=(oc == KC - 1),
                                             skip_group_check=True)
                    rstd2 = st_pool.tile([1, OWN], F32, tag="rstd2")
                    for i, (c0, c1) in enumerate(own_cuts):
                        nc.scalar.activation(out=rstd2[:, c0:c1], in_=ss2[i][:],
                                             func=mybir.ActivationFunctionType.Sqrt,
                                             bias=eps_t[:], scale=1.0 / DIM)
                    nc.vector.reciprocal(rstd2[:], rstd2[:])
                    nc.gpsimd.partition_broadcast(rstd2_b[:], rstd2[:])

        # ---- Phase 6: SwiGLU FFN + residual ----------------------------------
        for (c0, c1) in own_cuts:
            with ExitStack() as ph:
                t2_pool = ph.enter_context(tc.tile_pool(name="t2p", bufs=1))
                x2s_pool = ph.enter_context(tc.tile_pool(name="x2s", bufs=3))
                h_pool = ph.enter_context(tc.tile_pool(name="hbufp", bufs=1))
                wf_pool = ph.enter_context(tc.tile_pool(name="wfp", bufs=3))
                wfo_pool = ph.enter_context(tc.tile_pool(name="wfop", bufs=2))
                s_pool = ph.enter_context(tc.tile_pool(name="silp", bufs=3))
                ps_pool = ph.enter_context(
                    tc.tile_pool(name="f_ps", bufs=2, space="PSUM"))

                t2 = []
                for kc in range(KC):
                    xbs = x2s_pool.tile([P, 512], BF16, tag="xbs", name="xbs")
                    nc.sync.dma_start(out=xbs[:],
                                      in_=x2b_d[kc * P:(kc + 1) * P, c0:c1])
                    tt = t2_pool.tile([P, 512], BF16, tag=f"t2_{kc}",
                                      name=f"t2_{kc}")
                    nc.vector.tensor_mul(tt[:], xbs[:], rstd2_b[:, c0:c1])
                    t2.append(tt)

                hbuf = h_pool.tile([P, KF, 512], BF16)
                for oc in range(KF):
                    wfa_t = wf_pool.tile([P, KC, P], BF16, tag="wfa",
                                         name="wfa")
                    nc.sync.dma_start(out=wfa_t[:], in_=wfa_tl[oc])
                    ps1 = ps_pool.tile([P, 512], F32, tag="ps1", name="ps1")
                    for kc in range(KC):
                        nc.tensor.matmul(ps1[:], wfa_t[:, kc, :], t2[kc][:],
                                         start=(kc == 0), stop=(kc == KC - 1))
                    sil = s_pool.tile([P, 512], BF16, tag="sil", name="sil")
                    nc.scalar.activation(out=sil[:], in_=ps1[:],
                                         func=mybir.ActivationFunctionType.Silu)
                    wfc_t = wf_pool.tile([P, KC, P], BF16, tag="wfc",
                                         name="wfc")
                    nc.sync.dma_start(out=wfc_t[:], in_=wfc_tl[oc])
                    ps2 = ps_pool.tile([P, 512], F32, tag="ps2", name="ps2")
                    for kc in range(KC):
                        nc.tensor.matmul(ps2[:], wfc_t[:, kc, :], t2[kc][:],
                                         start=(kc == 0), stop=(kc == KC - 1))
                    nc.vector.tensor_mul(hbuf[:, oc, :], ps2[:], sil[:])

                for oc in range(KC):
                    wfo_t = wfo_pool.tile([P, KF, P], BF16, tag="wfo",
                                          name="wfo")
                    nc.sync.dma_start(out=wfo_t[:], in_=wfo_tl[oc])
                    ps = ps_pool.tile([P, 512], F32, tag="ps3", name="ps3")
                    for kc in range(KF):
                        nc.tensor.matmul(ps[:], wfo_t[:, kc, :],
                                         hbuf[:, kc, :],
                                         start=(kc == 0), stop=(kc == KF - 1))
                    x2s = x2s_pool.tile([P, 512], F32, tag="x2res",
                                        name="x2res")
                    nc.sync.dma_start(out=x2s[:],
                                      in_=x2T_d[oc * P:(oc + 1) * P, c0:c1])
                    yt = x2s_pool.tile([P, 512], F32, tag="yt", name="yt")
                    nc.vector.tensor_add(yt[:], ps[:], x2s[:])
                    nc.sync.dma_start(out=yT[oc * P:(oc + 1) * P, c0:c1],
                                      in_=yt[:])

    nc.compile()
    return nc


def _tile_w(wt, nkc, noc, ocw):
    """(din, dout) -> (dout//ocw, 128, din//128, ocw) so each [oc] is contiguous."""
    return np.ascontiguousarray(
        wt.reshape(nkc, P, noc, ocw).transpose(2, 1, 0, 3))


def _prep_inputs(x, wq, wk, wv, wo, last_k_init, last_v_init,
                 w_fc, w_fc_act, w_fc_out, g_mha, g_ffn):
    wq_t = ((wq * g_mha[None, :]).T).astype(BF)
    wk_t = ((wk * g_mha[None, :]).T).astype(BF)
    wv_t = ((wv * g_mha[None, :]).T).astype(BF)
    wo_t = wo.T.astype(BF)
    wfa_t = ((w_fc_act * g_ffn[None, :]).T).astype(BF)
    wfc_t = ((w_fc * g_ffn[None, :]).T).astype(BF)
    wfo_t = w_fc_out.T.astype(BF)

    shared = {
        "wq_tl": _tile_w(wq_t, KC, KC, P),
        "wk_tl": _tile_w(wk_t, KC, KC, P),
        "wv_tl": _tile_w(wv_t, KC, 4, 512),
        "wo_tl": _tile_w(wo_t, KC, KC, P),
        "wfa_tl": _tile_w(wfa_t, KC, KF, P),
        "wfc_tl": _tile_w(wfc_t, KC, KF, P),
        "wfo_tl": _tile_w(wfo_t, KF, KC, P),
    }

    # halo k/v for first-chunk cores, from last_k/v_init
    hk = np.zeros((W, H, HD), np.float32)
    hk[1:W] = last_k_init
    halo_kT0 = np.ascontiguousarray(hk.transpose(2, 1, 0)).astype(BF)  # (hd,h,j)
    hv = np.zeros((W, DIM), np.float32)
    hv[1:W] = last_v_init.reshape(W - 1, DIM)
    halo_v0 = hv.astype(BF)
    halo_kTz = np.zeros_like(halo_kT0)
    halo_vz = np.zeros_like(halo_v0)

    in_maps = []
    for c in range(NCORES):
        b, s = divmod(c * OWN, L)
        xe = np.zeros((EXT, DIM), np.float32)
        xe[W:] = x[b, s:s + OWN]
        if s > 0:
            xe[:W] = x[b, s - W:s]
        m = dict(shared)
        m["xT"] = np.ascontiguousarray(xe.T)
        m["halo_kT"] = halo_kT0 if s == 0 else halo_kTz
        m["halo_v"] = halo_v0 if s == 0 else halo_vz
        in_maps.append(m)
    return in_maps


def _run(inputs, trace=False):
    if "nc" not in _CACHE:
        _CACHE["nc"] = _build()
    nc = _CACHE["nc"]
    in_maps = _prep_inputs(**{k: np.asarray(v) for k, v in inputs.items()})
    res = run_bass_kernel_spmd(nc, in_maps, core_ids=list(range(NCORES)),
                               trace=trace)
    y = np.empty((B, L, DIM), np.float32)
    for c in range(NCORES):
        b, s = divmod(c * OWN, L)
        y[b, s:s + OWN] = res.results[c]["yT"].T
    return y, res


def kernel(**inputs):
    y, _ = _run(inputs, trace=False)
    return y


# revision 11
# speedup vs baseline: 91.5397x; 1.0022x over previous
"""Fused band-attention transformer block on 8 Trainium2 NeuronCores.

Sharding: data-parallel over tokens (B*L = 8192 -> 1024 own tokens/core,
plus a 128-token sequence halo so window attention needs no collectives;
batch 0 -> cores 0-3, batch 1 -> cores 4-7).
Per-core kernel computes rmsnorm -> QKV -> band attention -> O+residual ->
rmsnorm -> SwiGLU FFN -> residual, all activations feature-major (dim x tok),
matmuls in bf16 with f32 PSUM accumulation, residual stream in f32.
RMSNorm scales are folded into the matmul PSUM evictions (columns via a
partition-broadcast rstd row, V rows via a per-partition rstd column).
"""

from contextlib import ExitStack, nullcontext

import numpy as np
import ml_dtypes

import concourse.bacc as bacc
import concourse.bass as bass
import concourse.mybir as mybir
import concourse.tile as tile
from concourse.bass_utils import run_bass_kernel_spmd
from concourse.masks import make_identity

BF = ml_dtypes.bfloat16
F32 = mybir.dt.float32
BF16 = mybir.dt.bfloat16

B, L, DIM, H, W, DFF = 2, 4096, 2048, 16, 128, 8192
HD = DIM // H          # 128
P = 128
NCORES = 8
OWN = (B * L) // NCORES  # 1024 tokens per core
EXT = OWN + W            # 1152 with halo
KC = DIM // P            # 16 k-chunks over model dim
KF = DFF // P            # 64 k-chunks over ffn dim
NBLK = OWN // W          # 8 query blocks per core
MC = EXT // P            # 9 token tiles
EPS = 1e-6
SCALE = float(HD) ** -0.5

_CACHE = {}


def _build(n_loop=1):
    nc = bacc.Bacc("TRN2", target_bir_lowering=False, debug=False)

    xT = nc.dram_tensor("xT", [DIM, EXT], F32, kind="ExternalInput")
    halo_kT = nc.dram_tensor("halo_kT", [P, H, W], BF16, kind="ExternalInput")
    halo_v = nc.dram_tensor("halo_v", [W, DIM], BF16, kind="ExternalInput")
    wq_tl = nc.dram_tensor("wq_tl", [KC, P, KC, P], BF16, kind="ExternalInput")
    wk_tl = nc.dram_tensor("wk_tl", [KC, P, KC, P], BF16, kind="ExternalInput")
    wv_tl = nc.dram_tensor("wv_tl", [4, P, KC, 512], BF16, kind="ExternalInput")
    wo_tl = nc.dram_tensor("wo_tl", [KC, P, KC, P], BF16, kind="ExternalInput")
    wfa_tl = nc.dram_tensor("wfa_tl", [KF, P, KC, P], BF16, kind="ExternalInput")
    wfc_tl = nc.dram_tensor("wfc_tl", [KF, P, KC, P], BF16, kind="ExternalInput")
    wfo_tl = nc.dram_tensor("wfo_tl", [KC, P, KF, P], BF16, kind="ExternalInput")
    yT = nc.dram_tensor("yT", [DIM, OWN], F32, kind="ExternalOutput")

    ext_cuts = [(0, 512), (512, 1024), (1024, EXT)]
    own_cuts = [(0, 512), (512, 1024)]

    with tile.TileContext(nc) as tc, ExitStack() as top:
        dram = top.enter_context(tc.tile_pool(name="dram", bufs=1, space="DRAM"))
        x2T_d = dram.tile([DIM, OWN], F32, tag="x2T_d")
        x2b_d = dram.tile([DIM, OWN], BF16, tag="x2b_d")
        rstd1_d = dram.tile([1, EXT], F32, tag="rstd1_d")

        const = top.enter_context(tc.tile_pool(name="const", bufs=1))

        # band mask, additive: valid iff 1 <= j - p <= 128 (query p, window key j)
        mask = const.tile([P, 2 * W], F32)
        nc.gpsimd.memset(mask[:], 0.0)
        nc.gpsimd.affine_select(
            out=mask[:], in_=mask[:], compare_op=mybir.AluOpType.is_ge,
            fill=-1e4, base=-1, channel_multiplier=-1, pattern=[[1, 2 * W]])
        nc.gpsimd.affine_select(
            out=mask[:], in_=mask[:], compare_op=mybir.AluOpType.is_ge,
            fill=-1e4, base=W, channel_multiplier=1, pattern=[[-1, 2 * W]])

        ident = const.tile([P, P], BF16)
        make_identity(nc, ident[:])
        ones1 = const.tile([P, 1], BF16)
        nc.vector.memset(ones1[:], 1.0)
        eps_t = const.tile([1, 1], F32)
        nc.vector.memset(eps_t[:], EPS)

        rstd2_pool = top.enter_context(tc.tile_pool(name="rstd2p", bufs=1))
        rstd2_b = rstd2_pool.tile([P, OWN], F32, tag="rstd2_b")

        with (tc.For_i(0, n_loop, 1) if n_loop > 1 else nullcontext()):
            # n_loop > 1 is used only by bench.py to amortize dispatch overhead
            with ExitStack() as mha:  # Ph1..Ph5 buffers
                kv_pool = mha.enter_context(tc.tile_pool(name="kv", bufs=1))
                kT = kv_pool.tile([P, H, EXT], BF16, tag="kT")
                vv = kv_pool.tile([P, MC, DIM], BF16, tag="vv")
                xb_pool = mha.enter_context(tc.tile_pool(name="xbp", bufs=1))
                rs_pool = mha.enter_context(tc.tile_pool(name="rsp", bufs=1))
                rstd1_b = rs_pool.tile([P, EXT], F32, tag="rstd1_b")
                rstd1_c = rs_pool.tile([P, MC], F32, tag="rstd1_c")

                # ---- Phase 1: stream x, cast to bf16, rmsnorm1 stats ---------
                xb = []
                with ExitStack() as ph:
                    xf_pool = ph.enter_context(tc.tile_pool(name="xf", bufs=3))
                    xsq_pool = ph.enter_context(tc.tile_pool(name="xsq", bufs=3))
                    ss_ps = ph.enter_context(
                        tc.tile_pool(name="ss_ps", bufs=1, space="PSUM"))
                    ss = [ss_ps.tile([1, c1 - c0], F32, tag=f"ss{i}",
                                     name=f"ss{i}")
                          for i, (c0, c1) in enumerate(ext_cuts)]
                    for kc in range(KC):
                        xt = xf_pool.tile([P, EXT], F32, tag="xf", name="xf")
                        nc.sync.dma_start(out=xt[:],
                                          in_=xT[kc * P:(kc + 1) * P, :])
                        xbt = xb_pool.tile([P, EXT], BF16, tag=f"xb_{kc}",
                                           name=f"xb_{kc}")
                        nc.vector.tensor_copy(xbt[:], xt[:])
                        xb.append(xbt)
                        xsq = xsq_pool.tile([P, EXT], BF16, tag="xsq",
                                            name="xsq")
                        nc.vector.tensor_mul(xsq[:], xbt[:], xbt[:])
                        for i, (c0, c1) in enumerate(ext_cuts):
                            nc.tensor.matmul(ss[i][:], ones1[:], xsq[:, c0:c1],
                                             start=(kc == 0),
                                             stop=(kc == KC - 1),
                                             skip_group_check=True)
                    rstd1 = rs_pool.tile([1, EXT], F32, tag="rstd1")
                    for i, (c0, c1) in enumerate(ext_cuts):
                        nc.scalar.activation(
                            out=rstd1[:, c0:c1], in_=ss[i][:],
                            func=mybir.ActivationFunctionType.Sqrt,
                            bias=eps_t[:], scale=1.0 / DIM)
                    nc.vector.reciprocal(rstd1[:], rstd1[:])
                    nc.gpsimd.partition_broadcast(rstd1_b[:], rstd1[:])
                    # rstd as a (tok%128, tile) column view, via DRAM roundtrip
                    nc.sync.dma_start(out=rstd1_d[:], in_=rstd1[:])
                    nc.sync.dma_start(
                        out=rstd1_c[:],
                        in_=rstd1_d.rearrange("o (m p) -> (o p) m", p=P))

                # ---- Phase 2: V = (x.T @ Wv) * rstd  (tok x dim layout) ------
                with ExitStack() as ph:
                    wv_pool = ph.enter_context(tc.tile_pool(name="wv", bufs=2))
                    ps_pool = ph.enter_context(
                        tc.tile_pool(name="v_ps", bufs=4, space="PSUM"))
                    for ncol in range(4):
                        wv_t = wv_pool.tile([P, KC, 512], BF16, tag="wv",
                                            name="wv")
                        nc.sync.dma_start(out=wv_t[:], in_=wv_tl[ncol])
                        for mc in range(MC):
                            ps = ps_pool.tile([P, 512], F32, tag="vps",
                                              name="vps")
                            for kc in range(KC):
                                nc.tensor.matmul(
                                    ps[:], xb[kc][:, mc * P:(mc + 1) * P],
                                    wv_t[:, kc, :],
                                    start=(kc == 0), stop=(kc == KC - 1))
                            nc.vector.tensor_scalar_mul(
                                vv[:, mc, ncol * 512:(ncol + 1) * 512], ps[:],
                                rstd1_c[:, mc:mc + 1])
                    hv = wv_pool.tile([P, DIM], BF16, tag="halo_v")
                    nc.sync.dma_start(out=hv[:], in_=halo_v[:])
                    nc.vector.tensor_add(vv[:, 0, :], vv[:, 0, :], hv[:])

                # ---- Phase 3: K = (Wk.T @ x) * rstd (dim x tok layout) -------
                with ExitStack() as ph:
                    w_pool = ph.enter_context(tc.tile_pool(name="wkp", bufs=3))
                    ps_pool = ph.enter_context(
                        tc.tile_pool(name="k_ps", bufs=4, space="PSUM"))
                    for oc in range(KC):
                        wk_t = w_pool.tile([P, KC, P], BF16, tag="wk",
                                           name="wk")
                        nc.sync.dma_start(out=wk_t[:], in_=wk_tl[oc])
                        for (c0, c1) in ext_cuts:
                            ps = ps_pool.tile([P, 512], F32, tag="kps",
                                              name="kps")
                            for kc in range(KC):
                                nc.tensor.matmul(
                                    ps[:, :c1 - c0], wk_t[:, kc, :],
                                    xb[kc][:, c0:c1],
                                    start=(kc == 0), stop=(kc == KC - 1))
                            nc.vector.tensor_mul(kT[:, oc, c0:c1],
                                                 ps[:, :c1 - c0],
                                                 rstd1_b[:, c0:c1])
                    hk = w_pool.tile([P, H, W], BF16, tag="halo_k")
                    nc.sync.dma_start(out=hk[:], in_=halo_kT[:])
                    nc.vector.tensor_add(kT[:, :, 0:W], kT[:, :, 0:W], hk[:])

                # ---- Phase 4: per head: Q then band attention ----------------
                ao_pool = mha.enter_context(tc.tile_pool(name="aop", bufs=1))
                aoT = ao_pool.tile([P, KC, OWN], BF16, tag="aoT")
                with ExitStack() as ph:
                    w_pool = ph.enter_context(tc.tile_pool(name="wqp", bufs=3))
                    qh_pool = ph.enter_context(tc.tile_pool(name="qhp", bufs=2))
                    sm_pool = ph.enter_context(tc.tile_pool(name="smp", bufs=3))
                    p_pool = ph.enter_context(tc.tile_pool(name="ppp", bufs=3))
                    q_ps = ph.enter_context(
                        tc.tile_pool(name="q_ps", bufs=2, space="PSUM"))
                    sc_ps = ph.enter_context(
                        tc.tile_pool(name="sc_ps", bufs=2, space="PSUM"))
                    tp_ps = ph.enter_context(
                        tc.tile_pool(name="tp_ps", bufs=2, space="PSUM"))
                    av_ps = ph.enter_context(
                        tc.tile_pool(name="av_ps", bufs=2, space="PSUM"))
                    for h in range(H):
                        wq_t = w_pool.tile([P, KC, P], BF16, tag="wq",
                                           name="wq")
                        nc.sync.dma_start(out=wq_t[:], in_=wq_tl[h])
                        qh = qh_pool.tile([P, OWN], BF16, tag="qh", name="qh")
                        for (c0, c1) in own_cuts:
                            ps = q_ps.tile([P, 512], F32, tag="qps", name="qps")
                            for kc in range(KC):
                                nc.tensor.matmul(
                                    ps[:], wq_t[:, kc, :],
                                    xb[kc][:, W + c0:W + c1],
                                    start=(kc == 0), stop=(kc == KC - 1))
                            nc.vector.tensor_mul(qh[:, c0:c1], ps[:],
                                                 rstd1_b[:, W + c0:W + c1])
                        for n in range(NBLK):
                            sc = sc_ps.tile([P, 2 * W], F32, tag="sc",
                                            name="sc")
                            nc.tensor.matmul(sc[:], qh[:, n * W:(n + 1) * W],
                                             kT[:, h, n * W:n * W + 2 * W],
                                             start=True, stop=True)
                            sm = sm_pool.tile([P, 2 * W], F32, tag="sm",
                                              name="sm")
                            nc.vector.scalar_tensor_tensor(
                                out=sm[:], in0=sc[:], scalar=SCALE, in1=mask[:],
                                op0=mybir.AluOpType.mult,
                                op1=mybir.AluOpType.add)
                            pe = p_pool.tile([P, 2 * W], BF16, tag="pe",
                                             name="pe")
                            sumexp = sm_pool.tile([P, 1], F32, tag="sumexp",
                                                  name="sumexp")
                            nc.scalar.activation(
                                out=pe[:], in_=sm[:],
                                func=mybir.ActivationFunctionType.Exp,
                                accum_out=sumexp[:])
                            recip = sm_pool.tile([P, 1], F32, tag="recip",
                                                 name="recip")
                            nc.vector.reciprocal(recip[:], sumexp[:])
                            pn = p_pool.tile([P, 2 * W], BF16, tag="pn",
                                             name="pn")
                            nc.vector.tensor_scalar_mul(pn[:], pe[:], recip[:])
                            pT = p_pool.tile([P, 2, W], BF16, tag="pT",
                                             name="pT")
                            for c in range(2):
                                tp = tp_ps.tile([P, W], BF16, tag="tp",
                                                name="tp")
                                nc.tensor.transpose(
                                    tp[:], pn[:, c * W:(c + 1) * W], ident[:])
                                nc.vector.tensor_copy(pT[:, c, :], tp[:])
                            av = av_ps.tile([P, W], F32, tag="av", name="av")
                            for c in range(2):
                                nc.tensor.matmul(
                                    av[:], vv[:, n + c, h * HD:(h + 1) * HD],
                                    pT[:, c, :], start=(c == 0), stop=(c == 1))
                            nc.scalar.copy(aoT[:, h, n * W:(n + 1) * W], av[:])

                # ---- Phase 5: O proj + residual -> x2 (DRAM f32+bf16), ------
                # ---- fused rmsnorm2 stats -----------------------------------
                with ExitStack() as ph:
                    w_pool = ph.enter_context(tc.tile_pool(name="wop", bufs=3))
                    st_pool = ph.enter_context(tc.tile_pool(name="ost", bufs=3))
                    ps_pool = ph.enter_context(
                        tc.tile_pool(name="o_ps", bufs=3, space="PSUM"))
                    ss_ps = ph.enter_context(
                        tc.tile_pool(name="ss2_ps", bufs=1, space="PSUM"))
                    ss2 = [ss_ps.tile([1, 512], F32, tag=f"ss2_{i}",
                                      name=f"ss2_{i}") for i in range(2)]
                    for oc in range(KC):
                        wo_t = w_pool.tile([P, KC, P], BF16, tag="wo", name="wo")
                        nc.sync.dma_start(out=wo_t[:], in_=wo_tl[oc])
                        for i, (c0, c1) in enumerate(own_cuts):
                            ps = ps_pool.tile([P, 512], F32, tag="ops", name="ops")
                            for kc in range(KC):
                                nc.tensor.matmul(ps[:], wo_t[:, kc, :],
                                                 aoT[:, kc, c0:c1],
                                                 start=(kc == 0),
                                                 stop=(kc == KC - 1))
                            xres = st_pool.tile([P, 512], F32, tag="xres",
                                                name="xres")
                            nc.sync.dma_start(
                                out=xres[:],
                                in_=xT[oc * P:(oc + 1) * P, W + c0:W + c1])
                            x2 = st_pool.tile([P, 512], F32, tag="x2", name="x2")
                            nc.vector.tensor_add(x2[:], ps[:], xres[:])
                            nc.sync.dma_start(
                                out=x2T_d[oc * P:(oc + 1) * P, c0:c1], in_=x2[:])
                            xb2s = st_pool.tile([P, 512], BF16, tag="xb2s",
                                                name="xb2s")
                            nc.scalar.copy(xb2s[:], x2[:])
                            nc.sync.dma_start(
                                out=x2b_d[oc * P:(oc + 1) * P, c0:c1],
                                in_=xb2s[:])
                            xsq = st_pool.tile([P, 512], BF16, tag="xsq2",
                                               name="xsq2")
                            nc.vector.tensor_mul(xsq[:], x2[:], x2[:])
                            nc.tensor.matmul(ss2[i][:], ones1[:], xsq[:],
                                             start=(oc == 0), stop=(oc == KC - 1),
                                             skip_group_check=True)
                    rstd2 = st_pool.tile([1, OWN], F32, tag="rstd2")
                    for i, (c0, c1) in enumerate(own_cuts):
                        nc.scalar.activation(out=rstd2[:, c0:c1], in_=ss2[i][:],
                                             func=mybir.ActivationFunctionType.Sqrt,
                                             bias=eps_t[:], scale=1.0 / DIM)
                    nc.vector.reciprocal(rstd2[:], rstd2[:])
                    nc.gpsimd.partition_broadcast(rstd2_b[:], rstd2[:])

        # ---- Phase 6: SwiGLU FFN + residual ----------------------------------
        for (c0, c1) in own_cuts:
            with ExitStack() as ph:
                t2_pool = ph.enter_context(tc.tile_pool(name="t2p", bufs=1))
                x2s_pool = ph.enter_context(tc.tile_pool(name="x2s", bufs=3))
                h_pool = ph.enter_context(tc.tile_pool(name="hbufp", bufs=1))
                wf_pool = ph.enter_context(tc.tile_pool(name="wfp", bufs=3))
                wfo_pool = ph.enter_context(tc.tile_pool(name="wfop", bufs=2))
                s_pool = ph.enter_context(tc.tile_pool(name="silp", bufs=3))
                ps_pool = ph.enter_context(
                    tc.tile_pool(name="f_ps", bufs=2, space="PSUM"))

                t2 = []
                for kc in range(KC):
                    xbs = x2s_pool.tile([P, 512], BF16, tag="xbs", name="xbs")
                    nc.sync.dma_start(out=xbs[:],
                                      in_=x2b_d[kc * P:(kc + 1) * P, c0:c1])
                    tt = t2_pool.tile([P, 512], BF16, tag=f"t2_{kc}",
                                      name=f"t2_{kc}")
                    nc.vector.tensor_mul(tt[:], xbs[:], rstd2_b[:, c0:c1])
                    t2.append(tt)

                hbuf = h_pool.tile([P, KF, 512], BF16)
                for oc in range(KF):
                    wfa_t = wf_pool.tile([P, KC, P], BF16, tag="wfa",
                                         name="wfa")
                    nc.sync.dma_start(out=wfa_t[:], in_=wfa_tl[oc])
                    ps1 = ps_pool.tile([P, 512], F32, tag="ps1", name="ps1")
                    for kc in range(KC):
                        nc.tensor.matmul(ps1[:], wfa_t[:, kc, :], t2[kc][:],
                                         start=(kc == 0), stop=(kc == KC - 1))
                    sil = s_pool.tile([P, 512], BF16, tag="sil", name="sil")
                    nc.scalar.activation(out=sil[:], in_=ps1[:],
                                         func=mybir.ActivationFunctionType.Silu)
                    wfc_t = wf_pool.tile([P, KC, P], BF16, tag="wfc",
                                         name="wfc")
                    nc.sync.dma_start(out=wfc_t[:], in_=wfc_tl[oc])
                    ps2 = ps_pool.tile([P, 512], F32, tag="ps2", name="ps2")
                    for kc in range(KC):
                        nc.tensor.matmul(ps2[:], wfc_t[:, kc, :], t2[kc][:],
                                         start=(kc == 0), stop=(kc == KC - 1))
                    nc.vector.tensor_mul(hbuf[:, oc, :], ps2[:], sil[:])

                for oc in range(KC):
                    wfo_t = wfo_pool.tile([P, KF, P], BF16, tag="wfo",
                                          name="wfo")
                    nc.sync.dma_start(out=wfo_t[:], in_=wfo_tl[oc])
                    ps = ps_pool.tile([P, 512], F32, tag="ps3", name="ps3")
                    for kc in range(KF):
                        nc.tensor.matmul(ps[:], wfo_t[:, kc, :],
                                         hbuf[:, kc, :],
                                         start=(kc == 0), stop=(kc == KF - 1))
                    x2s = x2s_pool.tile([P, 512], F32, tag="x2res",
                                        name="x2res")
                    nc.sync.dma_start(out=x2s[:],
                                      in_=x2T_d[oc * P:(oc + 1) * P, c0:c1])
                    yt = x2s_pool.tile([P, 512], F32, tag="yt", name="yt")
                    nc.vector.tensor_add(yt[:], ps[:], x2s[:])
                    nc.sync.dma_start(out=yT[oc * P:(oc + 1) * P, c0:c1],
                                      in_=yt[:])

    nc.compile()
    return nc


def _tile_w(wt, nkc, noc, ocw):
    """(din, dout) -> (dout//ocw, 128, din//128, ocw) so each [oc] is contiguous."""
    return np.ascontiguousarray(
        wt.reshape(nkc, P, noc, ocw).transpose(2, 1, 0, 3))


def _prep_inputs(x, wq, wk, wv, wo, last_k_init, last_v_init,
                 w_fc, w_fc_act, w_fc_out, g_mha, g_ffn):
    wq_t = ((wq * g_mha[None, :]).T).astype(BF)
    wk_t = ((wk * g_mha[None, :]).T).astype(BF)
    wv_t = ((wv * g_mha[None, :]).T).astype(BF)
    wo_t = wo.T.astype(BF)
    wfa_t = ((w_fc_act * g_ffn[None, :]).T).astype(BF)
    wfc_t = ((w_fc * g_ffn[None, :]).T).astype(BF)
    wfo_t = w_fc_out.T.astype(BF)

    shared = {
        "wq_tl": _tile_w(wq_t, KC, KC, P),
        "wk_tl": _tile_w(wk_t, KC, KC, P),
        "wv_tl": _tile_w(wv_t, KC, 4, 512),
        "wo_tl": _tile_w(wo_t, KC, KC, P),
        "wfa_tl": _tile_w(wfa_t, KC, KF, P),
        "wfc_tl": _tile_w(wfc_t, KC, KF, P),
        "wfo_tl": _tile_w(wfo_t, KF, KC, P),
    }

    # halo k/v for first-chunk cores, from last_k/v_init
    hk = np.zeros((W, H, HD), np.float32)
    hk[1:W] = last_k_init
    halo_kT0 = np.ascontiguousarray(hk.transpose(2, 1, 0)).astype(BF)  # (hd,h,j)
    hv = np.zeros((W, DIM), np.float32)
    hv[1:W] = last_v_init.reshape(W - 1, DIM)
    halo_v0 = hv.astype(BF)
    halo_kTz = np.zeros_like(halo_kT0)
    halo_vz = np.zeros_like(halo_v0)

    in_maps = []
    for c in range(NCORES):
        b, s = divmod(c * OWN, L)
        xe = np.zeros((EXT, DIM), np.float32)
        xe[W:] = x[b, s:s + OWN]
        if s > 0:
            xe[:W] = x[b, s - W:s]
        m = dict(shared)
        m["xT"] = np.ascontiguousarray(xe.T)
        m["halo_kT"] = halo_kT0 if s == 0 else halo_kTz
        m["halo_v"] = halo_v0 if s == 0 else halo_vz
        in_maps.append(m)
    return in_maps


def _run(inputs, trace=False):
    if "nc" not in _CACHE:
        _CACHE["nc"] = _build()
    nc = _CACHE["nc"]
    in_maps = _prep_inputs(**{k: np.asarray(v) for k, v in inputs.items()})
    res = run_bass_kernel_spmd(nc, in_maps, core_ids=list(range(NCORES)),
                               trace=trace)
    y = np.empty((B, L, DIM), np.float32)
    for c in range(NCORES):
        b, s = divmod(c * OWN, L)
        y[b, s:s + OWN] = res.results[c]["yT"].T
    return y, res


def kernel(**inputs):
    y, _ = _run(inputs, trace=False)
    return y


# revision 14
# speedup vs baseline: 93.8053x; 1.0248x over previous
"""Fused band-attention transformer block on 8 Trainium2 NeuronCores.

Sharding: data-parallel over tokens (B*L = 8192 -> 1024 own tokens/core,
plus a 128-token sequence halo so window attention needs no collectives;
batch 0 -> cores 0-3, batch 1 -> cores 4-7).
Per-core kernel computes rmsnorm -> QKV -> band attention -> O+residual ->
rmsnorm -> SwiGLU FFN -> residual, all activations feature-major (dim x tok),
matmuls in bf16 with f32 PSUM accumulation, residual stream in f32.
RMSNorm scales are folded into the matmul PSUM evictions (columns via a
partition-broadcast rstd row, V rows via a per-partition rstd column).
"""

from contextlib import ExitStack, nullcontext

import numpy as np
import ml_dtypes

import concourse.bacc as bacc
import concourse.bass as bass
import concourse.mybir as mybir
import concourse.tile as tile
from concourse.bass_utils import run_bass_kernel_spmd
from concourse.masks import make_identity

BF = ml_dtypes.bfloat16
F32 = mybir.dt.float32
BF16 = mybir.dt.bfloat16

B, L, DIM, H, W, DFF = 2, 4096, 2048, 16, 128, 8192
HD = DIM // H          # 128
P = 128
NCORES = 8
OWN = (B * L) // NCORES  # 1024 tokens per core
EXT = OWN + W            # 1152 with halo
KC = DIM // P            # 16 k-chunks over model dim
KF = DFF // P            # 64 k-chunks over ffn dim
NBLK = OWN // W          # 8 query blocks per core
MC = EXT // P            # 9 token tiles
EPS = 1e-6
SCALE = float(HD) ** -0.5

_CACHE = {}


def _build(n_loop=1, wdma="sync", wf_bufs=3, wfo_bufs=2, split_fps=False,
           qkv_ps_bufs=4, x_split_q=False):
    nc = bacc.Bacc("TRN2", target_bir_lowering=False, debug=False)

    xT = nc.dram_tensor("xT", [DIM, EXT], F32, kind="ExternalInput")
    halo_kT = nc.dram_tensor("halo_kT", [P, H, W], BF16, kind="ExternalInput")
    halo_v = nc.dram_tensor("halo_v", [W, DIM], BF16, kind="ExternalInput")
    wq_tl = nc.dram_tensor("wq_tl", [KC, P, KC, P], BF16, kind="ExternalInput")
    wk_tl = nc.dram_tensor("wk_tl", [KC, P, KC, P], BF16, kind="ExternalInput")
    wv_tl = nc.dram_tensor("wv_tl", [4, P, KC, 512], BF16, kind="ExternalInput")
    wo_tl = nc.dram_tensor("wo_tl", [KC, P, KC, P], BF16, kind="ExternalInput")
    wfa_tl = nc.dram_tensor("wfa_tl", [KF, P, KC, P], BF16, kind="ExternalInput")
    wfc_tl = nc.dram_tensor("wfc_tl", [KF, P, KC, P], BF16, kind="ExternalInput")
    wfo_tl = nc.dram_tensor("wfo_tl", [KC, P, KF, P], BF16, kind="ExternalInput")
    yT = nc.dram_tensor("yT", [DIM, OWN], F32, kind="ExternalOutput")

    ext_cuts = [(0, 512), (512, 1024), (1024, EXT)]
    own_cuts = [(0, 512), (512, 1024)]
    wdma_start = nc.gpsimd.dma_start if wdma == "gpsimd" else nc.sync.dma_start

    with tile.TileContext(nc) as tc, ExitStack() as top:
        dram = top.enter_context(tc.tile_pool(name="dram", bufs=1, space="DRAM"))
        x2T_d = dram.tile([DIM, OWN], F32, tag="x2T_d")
        x2b_d = dram.tile([DIM, OWN], BF16, tag="x2b_d")
        rstd1_d = dram.tile([1, EXT], F32, tag="rstd1_d")

        const = top.enter_context(tc.tile_pool(name="const", bufs=1))

        # band mask, additive: valid iff 1 <= j - p <= 128 (query p, window key j)
        mask = const.tile([P, 2 * W], F32)
        nc.gpsimd.memset(mask[:], 0.0)
        nc.gpsimd.affine_select(
            out=mask[:], in_=mask[:], compare_op=mybir.AluOpType.is_ge,
            fill=-1e4, base=-1, channel_multiplier=-1, pattern=[[1, 2 * W]])
        nc.gpsimd.affine_select(
            out=mask[:], in_=mask[:], compare_op=mybir.AluOpType.is_ge,
            fill=-1e4, base=W, channel_multiplier=1, pattern=[[-1, 2 * W]])

        ident = const.tile([P, P], BF16)
        make_identity(nc, ident[:])
        ones1 = const.tile([P, 1], BF16)
        nc.vector.memset(ones1[:], 1.0)
        eps_t = const.tile([1, 1], F32)
        nc.vector.memset(eps_t[:], EPS)

        rstd2_pool = top.enter_context(tc.tile_pool(name="rstd2p", bufs=1))
        rstd2_b = rstd2_pool.tile([P, OWN], F32, tag="rstd2_b")

        with (tc.For_i(0, n_loop, 1) if n_loop > 1 else nullcontext()):
            # n_loop > 1 is used only by bench.py to amortize dispatch overhead
            with ExitStack() as mha:  # Ph1..Ph5 buffers
                kv_pool = mha.enter_context(tc.tile_pool(name="kv", bufs=1))
                kT = kv_pool.tile([P, H, EXT], BF16, tag="kT")
                vv = kv_pool.tile([P, MC, DIM], BF16, tag="vv")
                xb_pool = mha.enter_context(tc.tile_pool(name="xbp", bufs=1))
                rs_pool = mha.enter_context(tc.tile_pool(name="rsp", bufs=1))
                rstd1_b = rs_pool.tile([P, EXT], F32, tag="rstd1_b")
                rstd1_c = rs_pool.tile([P, MC], F32, tag="rstd1_c")

                # ---- Phase 1: stream x, cast to bf16, rmsnorm1 stats ---------
                xb = []
                with ExitStack() as ph:
                    xf_pool = ph.enter_context(tc.tile_pool(name="xf", bufs=3))
                    xsq_pool = ph.enter_context(tc.tile_pool(name="xsq", bufs=3))
                    ss_ps = ph.enter_context(
                        tc.tile_pool(name="ss_ps", bufs=1, space="PSUM"))
                    ss = [ss_ps.tile([1, c1 - c0], F32, tag=f"ss{i}",
                                     name=f"ss{i}")
                          for i, (c0, c1) in enumerate(ext_cuts)]
                    for kc in range(KC):
                        xt = xf_pool.tile([P, EXT], F32, tag="xf", name="xf")
                        eng = (nc.gpsimd if (x_split_q and kc % 2) else nc.sync)
                        eng.dma_start(out=xt[:],
                                      in_=xT[kc * P:(kc + 1) * P, :])
                        xbt = xb_pool.tile([P, EXT], BF16, tag=f"xb_{kc}",
                                           name=f"xb_{kc}")
                        nc.vector.tensor_copy(xbt[:], xt[:])
                        xb.append(xbt)
                        xsq = xsq_pool.tile([P, EXT], BF16, tag="xsq",
                                            name="xsq")
                        nc.vector.tensor_mul(xsq[:], xbt[:], xbt[:])
                        for i, (c0, c1) in enumerate(ext_cuts):
                            nc.tensor.matmul(ss[i][:], ones1[:], xsq[:, c0:c1],
                                             start=(kc == 0),
                                             stop=(kc == KC - 1),
                                             skip_group_check=True)
                    rstd1 = rs_pool.tile([1, EXT], F32, tag="rstd1")
                    for i, (c0, c1) in enumerate(ext_cuts):
                        nc.scalar.activation(
                            out=rstd1[:, c0:c1], in_=ss[i][:],
                            func=mybir.ActivationFunctionType.Sqrt,
                            bias=eps_t[:], scale=1.0 / DIM)
                    nc.vector.reciprocal(rstd1[:], rstd1[:])
                    nc.gpsimd.partition_broadcast(rstd1_b[:], rstd1[:])
                    # rstd as a (tok%128, tile) column view, via DRAM roundtrip
                    nc.sync.dma_start(out=rstd1_d[:], in_=rstd1[:])
                    nc.sync.dma_start(
                        out=rstd1_c[:],
                        in_=rstd1_d.rearrange("o (m p) -> (o p) m", p=P))

                # ---- Phase 2: V = (x.T @ Wv) * rstd  (tok x dim layout) ------
                with ExitStack() as ph:
                    wv_pool = ph.enter_context(tc.tile_pool(name="wv", bufs=2))
                    ps_pool = ph.enter_context(
                        tc.tile_pool(name="v_ps", bufs=qkv_ps_bufs, space="PSUM"))
                    for ncol in range(4):
                        wv_t = wv_pool.tile([P, KC, 512], BF16, tag="wv",
                                            name="wv")
                        wdma_start(out=wv_t[:], in_=wv_tl[ncol])
                        for mc in range(MC):
                            ps = ps_pool.tile([P, 512], F32, tag="vps",
                                              name="vps")
                            for kc in range(KC):
                                nc.tensor.matmul(
                                    ps[:], xb[kc][:, mc * P:(mc + 1) * P],
                                    wv_t[:, kc, :],
                                    start=(kc == 0), stop=(kc == KC - 1))
                            nc.vector.tensor_scalar_mul(
                                vv[:, mc, ncol * 512:(ncol + 1) * 512], ps[:],
                                rstd1_c[:, mc:mc + 1])
                    hv = wv_pool.tile([P, DIM], BF16, tag="halo_v")
                    nc.sync.dma_start(out=hv[:], in_=halo_v[:])
                    nc.vector.tensor_add(vv[:, 0, :], vv[:, 0, :], hv[:])

                # ---- Phase 3: K = (Wk.T @ x) * rstd (dim x tok layout) -------
                with ExitStack() as ph:
                    w_pool = ph.enter_context(tc.tile_pool(name="wkp", bufs=3))
                    ps_pool = ph.enter_context(
                        tc.tile_pool(name="k_ps", bufs=qkv_ps_bufs, space="PSUM"))
                    for oc in range(KC):
                        wk_t = w_pool.tile([P, KC, P], BF16, tag="wk",
                                           name="wk")
                        wdma_start(out=wk_t[:], in_=wk_tl[oc])
                        for (c0, c1) in ext_cuts:
                            ps = ps_pool.tile([P, 512], F32, tag="kps",
                                              name="kps")
                            for kc in range(KC):
                                nc.tensor.matmul(
                                    ps[:, :c1 - c0], wk_t[:, kc, :],
                                    xb[kc][:, c0:c1],
                                    start=(kc == 0), stop=(kc == KC - 1))
                            nc.vector.tensor_mul(kT[:, oc, c0:c1],
                                                 ps[:, :c1 - c0],
                                                 rstd1_b[:, c0:c1])
                    hk = w_pool.tile([P, H, W], BF16, tag="halo_k")
                    nc.sync.dma_start(out=hk[:], in_=halo_kT[:])
                    nc.vector.tensor_add(kT[:, :, 0:W], kT[:, :, 0:W], hk[:])

                # ---- Phase 4: per head: Q then band attention ----------------
                ao_pool = mha.enter_context(tc.tile_pool(name="aop", bufs=1))
                aoT = ao_pool.tile([P, KC, OWN], BF16, tag="aoT")
                with ExitStack() as ph:
                    w_pool = ph.enter_context(tc.tile_pool(name="wqp", bufs=3))
                    qh_pool = ph.enter_context(tc.tile_pool(name="qhp", bufs=2))
                    sm_pool = ph.enter_context(tc.tile_pool(name="smp", bufs=3))
                    p_pool = ph.enter_context(tc.tile_pool(name="ppp", bufs=3))
                    q_ps = ph.enter_context(
                        tc.tile_pool(name="q_ps", bufs=2, space="PSUM"))
                    sc_ps = ph.enter_context(
                        tc.tile_pool(name="sc_ps", bufs=2, space="PSUM"))
                    tp_ps = ph.enter_context(
                        tc.tile_pool(name="tp_ps", bufs=2, space="PSUM"))
                    av_ps = ph.enter_context(
                        tc.tile_pool(name="av_ps", bufs=2, space="PSUM"))
                    for h in range(H):
                        wq_t = w_pool.tile([P, KC, P], BF16, tag="wq",
                                           name="wq")
                        wdma_start(out=wq_t[:], in_=wq_tl[h])
                        qh = qh_pool.tile([P, OWN], BF16, tag="qh", name="qh")
                        for (c0, c1) in own_cuts:
                            ps = q_ps.tile([P, 512], F32, tag="qps", name="qps")
                            for kc in range(KC):
                                nc.tensor.matmul(
                                    ps[:], wq_t[:, kc, :],
                                    xb[kc][:, W + c0:W + c1],
                                    start=(kc == 0), stop=(kc == KC - 1))
                            nc.vector.tensor_mul(qh[:, c0:c1], ps[:],
                                                 rstd1_b[:, W + c0:W + c1])
                        for n in range(NBLK):
                            sc = sc_ps.tile([P, 2 * W], F32, tag="sc",
                                            name="sc")
                            nc.tensor.matmul(sc[:], qh[:, n * W:(n + 1) * W],
                                             kT[:, h, n * W:n * W + 2 * W],
                                             start=True, stop=True)
                            sm = sm_pool.tile([P, 2 * W], F32, tag="sm",
                                              name="sm")
                            nc.vector.scalar_tensor_tensor(
                                out=sm[:], in0=sc[:], scalar=SCALE, in1=mask[:],
                                op0=mybir.AluOpType.mult,
                                op1=mybir.AluOpType.add)
                            pe = p_pool.tile([P, 2 * W], BF16, tag="pe",
                                             name="pe")
                            sumexp = sm_pool.tile([P, 1], F32, tag="sumexp",
                                                  name="sumexp")
                            nc.scalar.activation(
                                out=pe[:], in_=sm[:],
                                func=mybir.ActivationFunctionType.Exp,
                                accum_out=sumexp[:])
                            recip = sm_pool.tile([P, 1], F32, tag="recip",
                                                 name="recip")
                            nc.vector.reciprocal(recip[:], sumexp[:])
                            pn = p_pool.tile([P, 2 * W], BF16, tag="pn",
                                             name="pn")
                            nc.vector.tensor_scalar_mul(pn[:], pe[:], recip[:])
                            pT = p_pool.tile([P, 2, W], BF16, tag="pT",
                                             name="pT")
                            for c in range(2):
                                tp = tp_ps.tile([P, W], BF16, tag="tp",
                                                name="tp")
                                nc.tensor.transpose(
                                    tp[:], pn[:, c * W:(c + 1) * W], ident[:])
                                nc.vector.tensor_copy(pT[:, c, :], tp[:])
                            av = av_ps.tile([P, W], F32, tag="av", name="av")
                            for c in range(2):
                                nc.tensor.matmul(
                                    av[:], vv[:, n + c, h * HD:(h + 1) * HD],
                                    pT[:, c, :], start=(c == 0), stop=(c == 1))
                            nc.scalar.copy(aoT[:, h, n * W:(n + 1) * W], av[:])

                # ---- Phase 5: O proj + residual -> x2 (DRAM f32+bf16), ------
                # ---- fused rmsnorm2 stats -----------------------------------
                with ExitStack() as ph:
                    w_pool = ph.enter_context(tc.tile_pool(name="wop", bufs=3))
                    st_pool = ph.enter_context(tc.tile_pool(name="ost", bufs=3))
                    ps_pool = ph.enter_context(
                        tc.tile_pool(name="o_ps", bufs=3, space="PSUM"))
                    ss_ps = ph.enter_context(
                        tc.tile_pool(name="ss2_ps", bufs=1, space="PSUM"))
                    ss2 = [ss_ps.tile([1, 512], F32, tag=f"ss2_{i}",
                                      name=f"ss2_{i}") for i in range(2)]
                    for oc in range(KC):
                        wo_t = w_pool.tile([P, KC, P], BF16, tag="wo", name="wo")
                        wdma_start(out=wo_t[:], in_=wo_tl[oc])
                        for i, (c0, c1) in enumerate(own_cuts):
                            ps = ps_pool.tile([P, 512], F32, tag="ops", name="ops")
                            for kc in range(KC):
                                nc.tensor.matmul(ps[:], wo_t[:, kc, :],
                                                 aoT[:, kc, c0:c1],
                                                 start=(kc == 0),
                                                 stop=(kc == KC - 1))
                            xres = st_pool.tile([P, 512], F32, tag="xres",
                                                name="xres")
                            nc.sync.dma_start(
                                out=xres[:],
                                in_=xT[oc * P:(oc + 1) * P, W + c0:W + c1])
                            x2 = st_pool.tile([P, 512], F32, tag="x2", name="x2")
                            nc.vector.tensor_add(x2[:], ps[:], xres[:])
                            nc.sync.dma_start(
                                out=x2T_d[oc * P:(oc + 1) * P, c0:c1], in_=x2[:])
                            xb2s = st_pool.tile([P, 512], BF16, tag="xb2s",
                                                name="xb2s")
                            nc.scalar.copy(xb2s[:], x2[:])
                            nc.sync.dma_start(
                                out=x2b_d[oc * P:(oc + 1) * P, c0:c1],
                                in_=xb2s[:])
                            xsq = st_pool.tile([P, 512], BF16, tag="xsq2",
                                               name="xsq2")
                            nc.vector.tensor_mul(xsq[:], x2[:], x2[:])
                            nc.tensor.matmul(ss2[i][:], ones1[:], xsq[:],
                                             start=(oc == 0), stop=(oc == KC - 1),
                                             skip_group_check=True)
                    rstd2 = st_pool.tile([1, OWN], F32, tag="rstd2")
                    for i, (c0, c1) in enumerate(own_cuts):
                        nc.scalar.activation(out=rstd2[:, c0:c1], in_=ss2[i][:],
                                             func=mybir.ActivationFunctionType.Sqrt,
                                             bias=eps_t[:], scale=1.0 / DIM)
                    nc.vector.reciprocal(rstd2[:], rstd2[:])
                    nc.gpsimd.partition_broadcast(rstd2_b[:], rstd2[:])

        # ---- Phase 6: SwiGLU FFN + residual ----------------------------------
        for (c0, c1) in own_cuts:
            with ExitStack() as ph:
                t2_pool = ph.enter_context(tc.tile_pool(name="t2p", bufs=1))
                x2s_pool = ph.enter_context(tc.tile_pool(name="x2s", bufs=3))
                h_pool = ph.enter_context(tc.tile_pool(name="hbufp", bufs=1))
                wf_pool = ph.enter_context(tc.tile_pool(name="wfp", bufs=wf_bufs))
                wfo_pool = ph.enter_context(tc.tile_pool(name="wfop", bufs=wfo_bufs))
                s_pool = ph.enter_context(tc.tile_pool(name="silp", bufs=3))
                if split_fps:
                    ps12_pool = ph.enter_context(
                        tc.tile_pool(name="f_ps12", bufs=2, space="PSUM"))
                    ps3_pool = ph.enter_context(
                        tc.tile_pool(name="f_ps3", bufs=4, space="PSUM"))
                else:
                    ps12_pool = ps3_pool = ph.enter_context(
                        tc.tile_pool(name="f_ps", bufs=2, space="PSUM"))

                t2 = []
                for kc in range(KC):
                    xbs = x2s_pool.tile([P, 512], BF16, tag="xbs", name="xbs")
                    nc.sync.dma_start(out=xbs[:],
                                      in_=x2b_d[kc * P:(kc + 1) * P, c0:c1])
                    tt = t2_pool.tile([P, 512], BF16, tag=f"t2_{kc}",
                                      name=f"t2_{kc}")
                    nc.vector.tensor_mul(tt[:], xbs[:], rstd2_b[:, c0:c1])
                    t2.append(tt)

                hbuf = h_pool.tile([P, KF, 512], BF16)
                for oc in range(KF):
                    wfa_t = wf_pool.tile([P, KC, P], BF16, tag="wfa",
                                         name="wfa")
                    wdma_start(out=wfa_t[:], in_=wfa_tl[oc])
                    ps1 = ps12_pool.tile([P, 512], F32, tag="ps1", name="ps1")
                    for kc in range(KC):
                        nc.tensor.matmul(ps1[:], wfa_t[:, kc, :], t2[kc][:],
                                         start=(kc == 0), stop=(kc == KC - 1))
                    sil = s_pool.tile([P, 512], BF16, tag="sil", name="sil")
                    nc.scalar.activation(out=sil[:], in_=ps1[:],
                                         func=mybir.ActivationFunctionType.Silu)
                    wfc_t = wf_pool.tile([P, KC, P], BF16, tag="wfc",
                                         name="wfc")
                    wdma_start(out=wfc_t[:], in_=wfc_tl[oc])
                    ps2 = ps12_pool.tile([P, 512], F32, tag="ps2", name="ps2")
                    for kc in range(KC):
                        nc.tensor.matmul(ps2[:], wfc_t[:, kc, :], t2[kc][:],
                                         start=(kc == 0), stop=(kc == KC - 1))
                    nc.vector.tensor_mul(hbuf[:, oc, :], ps2[:], sil[:])

                for oc in range(KC):
                    wfo_t = wfo_pool.tile([P, KF, P], BF16, tag="wfo",
                                          name="wfo")
                    wdma_start(out=wfo_t[:], in_=wfo_tl[oc])
                    ps = ps3_pool.tile([P, 512], F32, tag="ps3", name="ps3")
                    for kc in range(KF):
                        nc.tensor.matmul(ps[:], wfo_t[:, kc, :],
                                         hbuf[:, kc, :],
                                         start=(kc == 0), stop=(kc == KF - 1))
                    x2s = x2s_pool.tile([P, 512], F32, tag="x2res",
                                        name="x2res")
                    nc.sync.dma_start(out=x2s[:],
                                      in_=x2T_d[oc * P:(oc + 1) * P, c0:c1])
                    yt = x2s_pool.tile([P, 512], F32, tag="yt", name="yt")
                    nc.vector.tensor_add(yt[:], ps[:], x2s[:])
                    nc.sync.dma_start(out=yT[oc * P:(oc + 1) * P, c0:c1],
                                      in_=yt[:])

    nc.compile()
    return nc


def _tile_w(wt, nkc, noc, ocw):
    """(din, dout) -> (dout//ocw, 128, din//128, ocw) so each [oc] is contiguous."""
    return np.ascontiguousarray(
        wt.reshape(nkc, P, noc, ocw).transpose(2, 1, 0, 3))


def _prep_inputs(x, wq, wk, wv, wo, last_k_init, last_v_init,
                 w_fc, w_fc_act, w_fc_out, g_mha, g_ffn):
    wq_t = ((wq * g_mha[None, :]).T).astype(BF)
    wk_t = ((wk * g_mha[None, :]).T).astype(BF)
    wv_t = ((wv * g_mha[None, :]).T).astype(BF)
    wo_t = wo.T.astype(BF)
    wfa_t = ((w_fc_act * g_ffn[None, :]).T).astype(BF)
    wfc_t = ((w_fc * g_ffn[None, :]).T).astype(BF)
    wfo_t = w_fc_out.T.astype(BF)

    shared = {
        "wq_tl": _tile_w(wq_t, KC, KC, P),
        "wk_tl": _tile_w(wk_t, KC, KC, P),
        "wv_tl": _tile_w(wv_t, KC, 4, 512),
        "wo_tl": _tile_w(wo_t, KC, KC, P),
        "wfa_tl": _tile_w(wfa_t, KC, KF, P),
        "wfc_tl": _tile_w(wfc_t, KC, KF, P),
        "wfo_tl": _tile_w(wfo_t, KF, KC, P),
    }

    # halo k/v for first-chunk cores, from last_k/v_init
    hk = np.zeros((W, H, HD), np.float32)
    hk[1:W] = last_k_init
    halo_kT0 = np.ascontiguousarray(hk.transpose(2, 1, 0)).astype(BF)  # (hd,h,j)
    hv = np.zeros((W, DIM), np.float32)
    hv[1:W] = last_v_init.reshape(W - 1, DIM)
    halo_v0 = hv.astype(BF)
    halo_kTz = np.zeros_like(halo_kT0)
    halo_vz = np.zeros_like(halo_v0)

    in_maps = []
    for c in range(NCORES):
        b, s = divmod(c * OWN, L)
        xe = np.zeros((EXT, DIM), np.float32)
        xe[W:] = x[b, s:s + OWN]
        if s > 0:
            xe[:W] = x[b, s - W:s]
        m = dict(shared)
        m["xT"] = np.ascontiguousarray(xe.T)
        m["halo_kT"] = halo_kT0 if s == 0 else halo_kTz
        m["halo_v"] = halo_v0 if s == 0 else halo_vz
        in_maps.append(m)
    return in_maps


def _run(inputs, trace=False):
    if "nc" not in _CACHE:
        _CACHE["nc"] = _build()
    nc = _CACHE["nc"]
    in_maps = _prep_inputs(**{k: np.asarray(v) for k, v in inputs.items()})
    res = run_bass_kernel_spmd(nc, in_maps, core_ids=list(range(NCORES)),
                               trace=trace)
    y = np.empty((B, L, DIM), np.float32)
    for c in range(NCORES):
        b, s = divmod(c * OWN, L)
        y[b, s:s + OWN] = res.results[c]["yT"].T
    return y, res


def kernel(**inputs):
    y, _ = _run(inputs, trace=False)
    return y
